# revision 39
# baseline (speedup 1.0000x reference)
"""CRF loss (log-likelihood) kernel for Trainium2, 8 NeuronCores.

Strategy (v3):
  - Data-parallel: batch 512 sharded as 64 per core.
  - Denominator: exp-space forward+backward scans MERGED into one serial
    chain of 64x64 bf16 matmuls (block-diagonal weights [[exp(T),0],
    [0,exp(T)^T]]) + one DVE multiply per step; chains meet in the middle
    (384 steps).  The per-step DVE multiply reads BOTH operands from PSUM
    (the scan matmul output and the just-in-time pair transpose output),
    so it carries a single inline semaphore wait and no SBUF emission
    buffer is needed.
  - Emissions: X is DMA'd in fp32 chunks, exp-cast to bf16 by the scalar
    engine into a paired layout (pair r = [x_r | x_{767-r}], the bwd half
    time-reversed via negative-stride reads), then PE-transposed to PSUM
    2 steps ahead of consumption.
  - Renormalization every 8 steps: ones-matmul chain sums -> DVE
    reciprocal (logged in bf16 so applied == logged) -> PE outer-product
    broadcast -> one extra DVE multiply fused into the chain 4 steps
    later.
  - Numerator: ONE merged gpsimd ap_gather for emissions (8 batch-groups
    x 48 wrapped columns against a 12288-element per-partition table) and
    ONE for transitions, with all masking/reduction post-ops on gpsimd so
    the DVE scan stream is never blocked.  Wrapped-layout DMAs (64+64+64
    one-per-batch descriptors) are drained a few per scan step.
"""

import os
import sys

import numpy as np

for _p in ("/opt/trn_rl_repo", "/root/.axon_site/_ro/trn_rl_repo"):
    if os.path.isdir(_p) and _p not in sys.path:
        sys.path.insert(0, _p)

BS, T, NTAG = 512, 768, 32
NCORES = 8
B = BS // NCORES  # 64 batch per core
NPAIR = T // 2  # 384 merged scan steps
CH = 32         # pairs per pipeline chunk
NCHUNK = NPAIR // CH  # 12
RENORM = 8
REN_LAG = 4     # renorm measured at r, scale fused into step r+REN_LAG
TLEAD = 20      # transpose emitted TLEAD steps ahead of consumption

_state = {}
_DEBUG = False


def _emit(tc, nc, aps):
    import concourse.bass as bass
    from concourse import masks, mybir
    from concourse.tile import add_dep_helper

    f32 = mybir.dt.float32
    bf16 = mybir.dt.bfloat16
    i32 = mybir.dt.int32
    i16 = mybir.dt.int16
    AF = mybir.ActivationFunctionType
    ALU = mybir.AluOpType
    AX = mybir.AxisListType

    Xd, Yd, Td, Sd, Ed, Od = aps
    Xf = Xd.rearrange("b t j -> b (t j)")  # [64, 24576]

    from contextlib import ExitStack

    es = _state["es"] = ExitStack()
    persist = es.enter_context(tc.tile_pool(name="persist", bufs=1))
    xin = es.enter_context(tc.tile_pool(name="xin", bufs=2))
    xinb = es.enter_context(tc.tile_pool(name="xinb", bufs=3))
    sc_ps = es.enter_context(tc.tile_pool(name="sc_ps", bufs=3, space="PSUM"))
    tp_ps = es.enter_context(tc.tile_pool(name="tp_ps", bufs=3, space="PSUM"))
    sh_ps = es.enter_context(tc.tile_pool(name="sh_ps", bufs=1, space="PSUM"))
    rscr = es.enter_context(tc.tile_pool(name="rscr", bufs=2))

    # ---------------- constants ----------------
    ident = persist.tile([128, 64], bf16)   # identity at partitions 64-127
    masks.make_identity(nc, ident[64:128, :])
    id32 = persist.tile([32, 32], f32)
    masks.make_identity(nc, id32[:])

    ttab = persist.tile([32, 32], f32)
    nc.sync.dma_start(ttab[:], Td)
    # W64 = [[exp(T), 0], [0, exp(T)^T]]  (bf16, partitions 0-63)
    W64 = persist.tile([64, 64], bf16)
    nc.vector.memset(W64[:], 0.0)
    nc.scalar.activation(W64[0:32, 0:32], ttab[:], AF.Exp)
    # exp(T)^T: transpose at partition 0 (transpose outputs must land at
    # PSUM partition 0), exp there, then DMA to partitions 32-63
    tps2 = sh_ps.tile([32, 32], f32, tag="sp")
    nc.tensor.transpose(tps2[:], ttab[:], id32[:])
    exTT0 = persist.tile([32, 32], bf16)
    nc.scalar.activation(exTT0[:], tps2[:], AF.Exp)
    nc.sync.dma_start(W64[32:64, 32:64], exTT0[:])
    exTT32 = persist.tile([64, 32], bf16)
    nc.sync.dma_start(exTT32[32:64, :], exTT0[:])

    sraw = persist.tile([64, 1], f32)
    nc.sync.dma_start(sraw[0:32, :], Sd)
    nc.sync.dma_start(sraw[32:64, :], Ed)
    expSE = persist.tile([64, 1], f32)  # exp(start) rows 0-31, exp(end) 32-63
    nc.scalar.activation(expSE[:], sraw[:], AF.Exp)

    # sum-selector for renorm: col0 sums rows 0-31 (u), col1 rows 32-63 (y)
    ones2 = persist.tile([64, 2], bf16)
    nc.vector.memset(ones2[:], 0.0)
    nc.vector.memset(ones2[0:32, 0:1], 1.0)
    nc.vector.memset(ones2[32:64, 1:2], 1.0)
    # broadcast selector: row0 -> partitions 0-31, row1 -> partitions 32-63
    # (built via iota+compare; sub-32-aligned partition writes are illegal)
    itc = persist.tile([2, 64], i16)
    nc.gpsimd.iota(itc[:], pattern=[[1, 64]], base=0, channel_multiplier=0)
    itcs = persist.tile([2, 64], i16)
    nc.vector.tensor_scalar(itcs[:], itc[:], 5, None, op0=ALU.logical_shift_right)
    itcsf = persist.tile([2, 64], f32)
    nc.vector.tensor_copy(itcsf[:], itcs[:])
    itp = persist.tile([2, 1], i16)
    nc.gpsimd.iota(itp[:], pattern=[[0, 1]], base=0, channel_multiplier=1)
    itpf = persist.tile([2, 1], f32)
    nc.vector.tensor_copy(itpf[:], itp[:])
    SEL2T = persist.tile([2, 64], bf16)
    nc.vector.tensor_scalar(SEL2T[:], itcsf[:], itpf[:], None, op0=ALU.is_equal)
    ones2c = persist.tile([2, 1], f32)
    nc.vector.memset(ones2c[:], 1.0)
    ones32 = persist.tile([32, 1], f32)
    nc.vector.memset(ones32[:], 1.0)

    # ---------------- numerator setup -------
    NQ = T // 16  # 48
    Ywr = persist.tile([128, 8 * NQ], i32)
    # iota gives 1536*tau + 32*c (the merged-table column base per index)
    iow = persist.tile([128, 8 * NQ], i16)
    nc.gpsimd.iota(iow[:], pattern=[[NQ * 32, 8], [32, NQ]], base=0,
                   channel_multiplier=0)
    XWm = persist.tile([128, 8 * NQ * 32], f32)  # merged wrapped X table

    # flat Y -> pair indices -> DRAM bounce -> wrapped PIDX
    Yi = persist.tile([64, T], i32)
    nc.sync.dma_start(Yi[:], Yd)
    Yf_ = persist.tile([64, T], f32)
    nc.vector.tensor_copy(Yf_[:], Yi[:])
    NP = 800
    pidx = persist.tile([64, NP], f32)
    nc.vector.scalar_tensor_tensor(pidx[:, 0:767], Yf_[:, 0:767], 32.0,
                                   Yf_[:, 1:768], op0=ALU.mult, op1=ALU.add)
    nc.vector.tensor_scalar_add(pidx[:, 767:768], Yf_[:, 0:1], 1024.0)
    nc.vector.tensor_scalar_add(pidx[:, 768:769], Yf_[:, 767:768], 1056.0)
    nc.vector.memset(pidx[:, 769:800], 1088.0)
    pidx16 = persist.tile([64, NP], i16)
    nc.vector.tensor_copy(pidx16[:], pidx[:])
    dpool = es.enter_context(tc.tile_pool(name="dram", bufs=1, space="DRAM"))
    pd = dpool.tile([64, NP], i16)
    pdw = nc.sync.dma_start(pd[:], pidx16[:])
    NPC = NP // 16  # 50
    PIDX = persist.tile([128, 8 * NPC], i16)

    # wrap DMAs (one per batch element) drained a few per scan step,
    # split between the sync and scalar-engine DGE queues so neither the
    # scan-critical X chunk loads nor the EX block copies get crowded out
    sync_q = []
    act_q = []
    for b in range(B):
        g, tau = b % 8, b // 8

        def _ywr(b=b, g=g, tau=tau):
            nc.sync.dma_start(
                Ywr[16 * g:16 * g + 16, NQ * tau:NQ * tau + NQ],
                Yd[b:b + 1, :].rearrange("a (c p) -> a p c", p=16),
            )
        sync_q.append(_ywr)
    for b in range(B):
        g, tau = b % 8, b // 8

        def _xwm(b=b, g=g, tau=tau, on_act=(b % 2 == 0)):
            eng = nc.scalar if on_act else nc.sync
            eng.dma_start(
                XWm[16 * g:16 * g + 16, 1536 * tau:1536 * tau + 1536],
                Xf[b:b + 1, :].rearrange("a (q p j) -> a p q j", p=16, j=32))
        (act_q if b % 2 == 0 else sync_q).append(_xwm)
    for b in range(B):
        g, tau = b % 8, b // 8

        def _pidxw(b=b, g=g, tau=tau):
            wi = nc.sync.dma_start(
                PIDX[16 * g:16 * g + 16, NPC * tau:NPC * tau + NPC],
                pd[b:b + 1, :].rearrange("a (c p) -> a p c", p=16),
            )
            add_dep_helper(wi.ins, pdw.ins, sync=True,
                           reason="wrap read waits for dram roundtrip write")
        sync_q.append(_pidxw)

    def build_eidx():
        Ywrf = persist.tile([128, 8 * NQ], f32)
        nc.vector.tensor_copy(Ywrf[:], Ywr[:])
        iowf = persist.tile([128, 8 * NQ], f32)
        nc.vector.tensor_copy(iowf[:], iow[:])
        eidxf = persist.tile([128, 8 * NQ], f32)
        nc.vector.tensor_add(eidxf[:], iowf[:], Ywrf[:])
        EIDX = persist.tile([128, 8 * NQ], i16)
        nc.vector.tensor_copy(EIDX[:], eidxf[:])
        return EIDX

    # table: [T flat 1024 | start 32 | end 32 | zeros 4] on 128 partitions
    TTAB = persist.tile([128, 1092], f32)
    nc.gpsimd.memset(TTAB[0:1, :], 0.0)
    nc.sync.dma_start(TTAB[0:1, 0:1024], Td.rearrange("i j -> (i j)"))
    nc.sync.dma_start(TTAB[0:1, 1024:1056], Sd)
    nc.sync.dma_start(TTAB[0:1, 1056:1088], Ed)
    nc.gpsimd.partition_broadcast(TTAB[:], TTAB[0:1, :])

    # static diag mask for the emission gather: [p, k] = (k%16 == p%16)
    iok = persist.tile([128, T], i16)
    nc.gpsimd.iota(iok[:], pattern=[[0, NQ], [1, 16]], base=0, channel_multiplier=0)
    iokf = persist.tile([128, T], f32)
    nc.vector.tensor_copy(iokf[:], iok[:])
    iop = persist.tile([128, 1], i16)
    nc.gpsimd.iota(iop[:], pattern=[[0, 1]], base=0, channel_multiplier=1)
    pmod = persist.tile([128, 1], i16)
    nc.vector.tensor_scalar(pmod[:], iop[:], 15, None, op0=ALU.bitwise_and)
    pmodf = persist.tile([128, 1], f32)
    nc.vector.tensor_copy(pmodf[:], pmod[:])
    dmask = persist.tile([128, T], f32)
    nc.vector.tensor_scalar(dmask[:], iokf[:], pmodf[:], None, op0=ALU.is_equal)

    # selection matrices for the per-group combine matmuls
    iog = persist.tile([128, 8], i16)
    nc.gpsimd.iota(iog[:], pattern=[[1, 8]], base=0, channel_multiplier=0)
    iogf = persist.tile([128, 8], f32)
    nc.vector.tensor_copy(iogf[:], iog[:])
    pdiv = persist.tile([128, 1], i16)
    nc.vector.tensor_scalar(pdiv[:], iop[:], 4, None, op0=ALU.logical_shift_right)
    pdivf = persist.tile([128, 1], f32)
    nc.vector.tensor_copy(pdivf[:], pdiv[:])
    SELe = persist.tile([128, 8], f32)
    nc.vector.tensor_scalar(SELe[:], iogf[:], pdivf[:], None, op0=ALU.is_equal)
    SELt = persist.tile([128, 8], f32)
    nc.vector.tensor_scalar_mul(SELt[:], SELe[:], 1.0 / 16.0)

    empart = persist.tile([128, 8], f32)
    tpart = persist.tile([128, 8], f32)

    # ---------------- emission pipeline ----------------
    # paired bf16 exp(X) chunk staging: pair r cols [64r',64r'+32) =
    # exp(x_r), [64r'+32,64r'+64) = exp(x_{767-r}) (r' chunk-local);
    # partitions 64-127 (batch)
    CHT = {}  # chunk i -> staged paired bf16 tile

    def chunk_prep_head(i):
        # DMA + exp-cast for chunk i (pairs 32i..32i+31)
        lo = CH * i
        xp = xinb.tile([128, CH * 64], bf16)
        xpv = xp[64:128, :].rearrange("p (r h j) -> p r h j", h=2, j=32)
        CHT[i] = xp
        xf = xin.tile([128, CH * 32], f32)
        nc.sync.dma_start(xf[64:128, :], Xf[:, 32 * lo:32 * (lo + CH)])
        nc.scalar.activation(
            xpv[:, :, 0, :],
            xf[64:128, :].rearrange("p (t j) -> p t j", j=32), AF.Exp)
        xb = xin.tile([128, CH * 32], f32)
        blo = T - lo - CH  # covers t' = blo .. blo+CH-1 (= 767-r desc)
        nc.sync.dma_start(xb[64:128, :], Xf[:, 32 * blo:32 * (blo + CH)])
        # reversed read: pair r = 767-t' ascends as t' descends
        nc.scalar.activation(
            xpv[:, :, 1, :],
            xb[64:128, :].rearrange("p (t j) -> p t j", j=32)[:, ::-1, :],
            AF.Exp)

    # transposes land in 16-pair PSUM block tiles; one ACT copy per full
    # block moves it to the SBUF EX buffer (so scan muls carry at most one
    # extra semaphore wait per 16 steps)
    EX = persist.tile([64, 64 * NPAIR], bf16)
    BLK = 16
    TPB = {}  # block index -> PSUM block tile

    def emit_pair(r):
        b, k = r // BLK, r % BLK
        if k == 0:
            TPB[b] = tp_ps.tile([64, 64 * BLK], bf16, tag="tp", name=f"tpb_{b}")
        xp = CHT[r // CH]
        rl = r % CH
        nc.tensor.transpose(TPB[b][:, 64 * k:64 * k + 64],
                            xp[64:128, 64 * rl:64 * rl + 64],
                            ident[64:128, :])
        if k == BLK - 1:
            nc.scalar.copy(EX[:, 64 * BLK * b:64 * BLK * (b + 1)], TPB[b][:])
            del TPB[b]
            for _ in range(4):
                if act_q:
                    act_q.pop(0)()

    # ---------------- numerator gathers (merged, all on gpsimd) -------
    # one shared output buffer for both merged gathers (sequential on the
    # gpsimd queue, WAR-serialized by tile deps)
    gtile = persist.tile([128, 8 * NP], f32)

    # ONE shared f32 output buffer, time-shared: emission gather ->
    # emission accums (DVE, issued late enough that the data is certainly
    # ready) -> transition gather (WAR-ordered by tile deps) -> transition
    # accums.  The DVE accums fill scan idle instead of blocking it.

    def gather_emis(EIDX):
        nc.gpsimd.ap_gather(gtile[:, 0:8 * T], XWm[:], EIDX[:],
                            channels=128, num_elems=8 * NQ * 32, d=1,
                            num_idxs=8 * T)

    def gather_trans():
        nc.gpsimd.ap_gather(gtile[:], TTAB[:], PIDX[:],
                            channels=128, num_elems=1092, d=1, num_idxs=8 * NP)

    def accum_tau(k):
        if k < 8:
            tau = k
            sl = gtile[:, T * tau:T * tau + T]
            nc.vector.scalar_tensor_tensor(sl, sl, 1.0, dmask[:],
                                           op0=ALU.bypass, op1=ALU.mult,
                                           accum_out=empart[:, tau:tau + 1])
        else:
            tau = k - 8
            sl = gtile[:, NP * tau:NP * tau + NP]
            nc.vector.tensor_scalar(sl, sl, 1.0, 0.0, op0=ALU.mult,
                                    op1=ALU.add,
                                    accum_out=tpart[:, tau:tau + 1])

    # ---------------- the scan ----------------
    # +1 slot: a final renorm of the last state keeps the combine's
    # chain-product inside the scalar engine's Ln range
    NREN = len(range(RENORM, NPAIR - REN_LAG, RENORM)) + 1
    rst = persist.tile([2, 64 * NREN], bf16)
    ren_slot = [0]

    U2 = [persist.tile([64, 64], bf16, name=f"u2_{k}") for k in range(3)]

    def st(r):
        return U2[r % 3]

    def renorm_a(r):
        # measure sums of both chains on state r.  bf16 scales so the
        # applied factor and the Ln-logged factor are bit-identical.
        sp = sh_ps.tile([2, 64], f32, tag="sp")
        nc.tensor.matmul(sp[:], ones2[:], st(r)[:], tile_position=(0, 0))
        srec = rscr.tile([2, 64], f32)
        nc.vector.reciprocal_approx_fast(srec[:], sp[:])
        m = ren_slot[0]
        ren_slot[0] += 1
        rsl = rst[:, 64 * m:64 * m + 64]
        nc.vector.tensor_copy(rsl, srec[:])
        return rsl

    def renorm_b(rsl):
        rb = sh_ps.tile([64, 64], f32, tag="rb")
        nc.tensor.matmul(rb[:], SEL2T[:], rsl, tile_position=(0, 0))
        return rb

    # prologue: chunk 0 prepped with the first transposes, chunk 1 queued
    chunk_prep_head(0)
    for rr in range(TLEAD + 1):
        emit_pair(rr)
    chunk_prep_head(1)

    # u_0 = exp(start) * ex_0 ; y_767 = exp(end) * ex_767
    nc.vector.tensor_scalar_mul(st(0)[:], EX[:, 0:64], expSE[:, 0:1])

    pend_a = None   # rsl awaiting broadcast
    pend_b = {}     # r -> rb PSUM tile to fuse at step r
    for r in range(1, NPAIR):
        i = r // CH
        if r % CH == 0 and i + 1 < NCHUNK:
            chunk_prep_head(i + 1)
        if r % 8 == 5 and sync_q:
            for _ in range(12):
                if sync_q:
                    sync_q.pop(0)()
        if r == 130:
            eidx_box = build_eidx()
        if r == 135:
            gather_emis(eidx_box)
        if r >= 200 and (r - 200) % 2 == 0 and (r - 200) // 2 < 8:
            accum_tau((r - 200) // 2)
        if r == 222:
            gather_trans()
        if r >= 320 and (r - 320) % 2 == 0 and (r - 320) // 2 < 8:
            accum_tau(8 + (r - 320) // 2)
        if r + TLEAD < NPAIR:
            emit_pair(r + TLEAD)
        vp = sc_ps.tile([64, 64], f32, tag="sc")
        nc.tensor.matmul(vp[:], W64[:], st(r - 1)[:], tile_position=(0, 0))
        exs = EX[:, 64 * r:64 * r + 64]
        rb = pend_b.pop(r, None)
        if rb is None:
            nc.vector.tensor_mul(st(r)[:], vp[:], exs)
        else:
            u2t = rscr.tile([64, 64], f32)
            nc.vector.tensor_mul(u2t[:], vp[:], exs)
            nc.vector.tensor_mul(st(r)[:], u2t[:], rb[:])
        if pend_a is not None:
            pend_b[r + REN_LAG - 1] = renorm_b(pend_a)
            pend_a = None
        if r % RENORM == 0 and r + REN_LAG < NPAIR and ren_slot[0] < NREN - 1:
            pend_a = renorm_a(r)

    # ---------------- combine: Z = u_383^T exp(T) y_384 ----------------
    # final renorm: both chains scaled to unit sum (and logged) so the
    # product stays well inside the Ln table range
    rslF = renorm_a(NPAIR - 1)
    rbF = renorm_b(rslF)
    last = persist.tile([64, 64], bf16)
    nc.vector.tensor_mul(last[:], st(NPAIR - 1)[:], rbF[:])
    w383 = sc_ps.tile([32, 64], f32, tag="sc")
    nc.tensor.matmul(w383[:], exTT32[32:64, :], last[32:64, :],
                     tile_position=(32, 0))
    q = persist.tile([32, 64], f32)
    nc.vector.tensor_mul(q[:], w383[:], last[0:32, :])
    combo = sh_ps.tile([1, 64], f32, tag="sp")
    nc.tensor.matmul(combo[:], ones32[:], q[:], tile_position=(0, 0))

    nump = sh_ps.tile([1, 64], f32, tag="rb")
    for tau in range(8):
        sl = nump[0:1, 8 * tau:8 * tau + 8]
        nc.tensor.matmul(sl, empart[:, tau:tau + 1], SELe[:], start=True,
                         stop=False, tile_position=(0, 0))
        nc.tensor.matmul(sl, tpart[:, tau:tau + 1], SELt[:], start=False,
                         stop=True, tile_position=(0, 0))

    # ---------------- final assembly ----------------
    lncombo = persist.tile([1, 64], f32)
    nc.scalar.activation(lncombo[:], combo[:], AF.Ln)
    lnr = persist.tile([2, 64 * NREN], f32)
    nc.scalar.activation(lnr[:], rst[:], AF.Ln)
    lnrsum = persist.tile([2, 64], f32)
    nc.vector.tensor_reduce(lnrsum[:], lnr[:].rearrange("p (m b) -> p b m", b=64),
                            AX.X, ALU.add)
    lnboth = sh_ps.tile([1, 64], f32, tag="sp")
    nc.tensor.matmul(lnboth[:], ones2c[:], lnrsum[:], tile_position=(0, 0))
    f1 = persist.tile([1, 64], f32)
    nc.vector.tensor_sub(f1[:], nump[:], lncombo[:])
    f2 = persist.tile([1, 64], f32)
    nc.vector.tensor_add(f2[:], f1[:], lnboth[:])
    nc.sync.dma_start(Od, f2[:])

    if _DEBUG:
        def dout(name, ap):
            d = nc.dram_tensor(name, list(ap.shape), ap.dtype,
                               kind="ExternalOutput").ap()
            nc.sync.dma_start(d, ap)
        dout("d_u2", last[:]); dout("d_rst", rst[:]); dout("d_q", q[:])
        dout("d_empart", empart[:]); dout("d_tpart", tpart[:])
        dout("d_f1", f1[:]); dout("d_lnrsum", lnrsum[:])

    es.close()


def _build():
    import concourse.tile as tile
    from concourse import bacc, mybir

    f32 = mybir.dt.float32
    i32 = mybir.dt.int32

    nc = bacc.Bacc("TRN2", target_bir_lowering=False, debug=False,
                   enable_asserts=False, num_devices=NCORES)
    Xd = nc.dram_tensor("x", [B, T, NTAG], f32, kind="ExternalInput").ap()
    Yd = nc.dram_tensor("y", [B, T], i32, kind="ExternalInput").ap()
    Td = nc.dram_tensor("t", [NTAG, NTAG], f32, kind="ExternalInput").ap()
    Sd = nc.dram_tensor("s", [NTAG], f32, kind="ExternalInput").ap()
    Ed = nc.dram_tensor("e", [NTAG], f32, kind="ExternalInput").ap()
    Od = nc.dram_tensor("o", [B], f32, kind="ExternalOutput").ap()
    with tile.TileContext(nc) as tc:
        _emit(tc, nc, (Xd, Yd, Td, Sd, Ed, Od))
    nc.compile()
    return nc


def _numpy_fallback(X, Y, mask, transition, start_trans, end_trans):
    X = np.asarray(X, np.float64)
    Y = np.asarray(Y, np.int64)
    m = np.asarray(mask, bool)
    Tm = np.asarray(transition, np.float64)
    st = np.asarray(start_trans, np.float64)
    en = np.asarray(end_trans, np.float64)
    bs, sl, nt = X.shape
    rb = np.arange(bs)
    mf = m.astype(np.float64)
    score = st[Y[:, 0]] + X[rb, 0, Y[:, 0]]
    emit = np.take_along_axis(X[:, 1:], Y[:, 1:, None], axis=2)[..., 0]
    tr = Tm[Y[:, :-1], Y[:, 1:]]
    score = score + np.sum((tr + emit) * mf[:, 1:], axis=1)
    each_len = m.sum(1).astype(np.int64)
    last_tag = Y[rb, each_len - 1]
    score = score + en[last_tag] * mf[rb, each_len - 1]
    alpha = st[None, :] + X[:, 0]
    for t in range(1, sl):
        s = alpha[:, :, None] + Tm[None] + X[:, t][:, None, :]
        mx = s.max(1)
        new = mx + np.log(np.exp(s - mx[:, None, :]).sum(1))
        alpha = np.where(m[:, t][:, None], new, alpha)
    mx = (alpha + en).max(1)
    logZ = mx + np.log(np.exp(alpha + en - mx[:, None]).sum(1))
    return (score - logZ).astype(np.float32)


def kernel(X, Y, mask, transition, start_trans, end_trans):
    X = np.ascontiguousarray(np.asarray(X, dtype=np.float32))
    Yc = np.ascontiguousarray(np.asarray(Y).astype(np.int32))
    Tm = np.ascontiguousarray(np.asarray(transition, dtype=np.float32))
    st = np.ascontiguousarray(np.asarray(start_trans, dtype=np.float32))
    en = np.ascontiguousarray(np.asarray(end_trans, dtype=np.float32))
    mk = np.asarray(mask)

    if X.shape != (BS, T, NTAG) or not bool(mk.all()):
        return _numpy_fallback(X, Y, mask, transition, start_trans, end_trans)

    from concourse import bass_utils

    if "nc" not in _state:
        _state["nc"] = _build()
    nc = _state["nc"]

    in_maps = []
    for c in range(NCORES):
        sl = slice(B * c, B * (c + 1))
        in_maps.append({"x": X[sl], "y": Yc[sl], "t": Tm, "s": st, "e": en})
    res = bass_utils.run_bass_kernel_spmd(nc, in_maps, core_ids=list(range(NCORES)))
    out = np.concatenate([res.results[c]["o"] for c in range(NCORES)])
    return out.astype(np.float32)


if __name__ == "__main__":
    sys.path.insert(0, "/root/problem")
    import reference

    inputs = reference.setup_inputs()
    inputs = {k: np.asarray(v) for k, v in inputs.items()}
    exp = np.asarray(reference.reference(**inputs))
    act = kernel(**inputs)
    err = np.abs(act - exp) / np.maximum(np.abs(exp), 1e-6)
    print("max rel err:", err.max(), "mean:", err.mean())


# revision 44
# speedup vs baseline: 1.3572x; 1.3572x over previous
"""CRF loss (log-likelihood) kernel for Trainium2, 8 NeuronCores.

Strategy (v3):
  - Data-parallel: batch 512 sharded as 64 per core.
  - Denominator: exp-space forward+backward scans MERGED into one serial
    chain of 64x64 bf16 matmuls (block-diagonal weights [[exp(T),0],
    [0,exp(T)^T]]) + one DVE multiply per step; chains meet in the middle
    (384 steps).  The per-step DVE multiply reads BOTH operands from PSUM
    (the scan matmul output and the just-in-time pair transpose output),
    so it carries a single inline semaphore wait and no SBUF emission
    buffer is needed.
  - Emissions: X is DMA'd in fp32 chunks, exp-cast to bf16 by the scalar
    engine into a paired layout (pair r = [x_r | x_{767-r}], the bwd half
    time-reversed via negative-stride reads), then PE-transposed to PSUM
    2 steps ahead of consumption.
  - Renormalization every 8 steps: ones-matmul chain sums -> DVE
    reciprocal (logged in bf16 so applied == logged) -> PE outer-product
    broadcast -> one extra DVE multiply fused into the chain 4 steps
    later.
  - Numerator: ONE merged gpsimd ap_gather for emissions (8 batch-groups
    x 48 wrapped columns against a 12288-element per-partition table) and
    ONE for transitions, with all masking/reduction post-ops on gpsimd so
    the DVE scan stream is never blocked.  Wrapped-layout DMAs (64+64+64
    one-per-batch descriptors) are drained a few per scan step.
"""

import os
import sys

import numpy as np

for _p in ("/opt/trn_rl_repo", "/root/.axon_site/_ro/trn_rl_repo"):
    if os.path.isdir(_p) and _p not in sys.path:
        sys.path.insert(0, _p)

BS, T, NTAG = 512, 768, 32
NCORES = 8
B = BS // NCORES  # 64 batch per core
NPAIR = T // 2  # 384 merged scan steps
CH = 32         # pairs per pipeline chunk
NCHUNK = NPAIR // CH  # 12
RENORM = 8
REN_LAG = 4     # renorm measured at r, scale fused into step r+REN_LAG
TLEAD = 20      # transpose emitted TLEAD steps ahead of consumption

_state = {}
_DEBUG = False


def _emit(tc, nc, aps):
    import concourse.bass as bass
    from concourse import masks, mybir
    from concourse.tile import add_dep_helper

    f32 = mybir.dt.float32
    bf16 = mybir.dt.bfloat16
    i32 = mybir.dt.int32
    i16 = mybir.dt.int16
    AF = mybir.ActivationFunctionType
    ALU = mybir.AluOpType
    AX = mybir.AxisListType

    Xd, Yd, Td, Sd, Ed, Od = aps
    Xf = Xd.rearrange("b t j -> b (t j)")  # [64, 24576]

    from contextlib import ExitStack

    es = _state["es"] = ExitStack()
    persist = es.enter_context(tc.tile_pool(name="persist", bufs=1))
    xin = es.enter_context(tc.tile_pool(name="xin", bufs=2))
    xinb = es.enter_context(tc.tile_pool(name="xinb", bufs=3))
    sc_ps = es.enter_context(tc.tile_pool(name="sc_ps", bufs=3, space="PSUM"))
    tp_ps = es.enter_context(tc.tile_pool(name="tp_ps", bufs=3, space="PSUM"))
    sh_ps = es.enter_context(tc.tile_pool(name="sh_ps", bufs=1, space="PSUM"))
    rscr = es.enter_context(tc.tile_pool(name="rscr", bufs=2))

    # ---------------- constants ----------------
    ident = persist.tile([128, 64], bf16)   # identity at partitions 64-127
    masks.make_identity(nc, ident[64:128, :])
    identF = persist.tile([128, 64], f32)   # f32 identity for fp32 transposes
    masks.make_identity(nc, identF[64:128, :])
    id32 = persist.tile([32, 32], f32)
    masks.make_identity(nc, id32[:])

    ttab = persist.tile([32, 32], f32)
    nc.sync.dma_start(ttab[:], Td)
    # W64 = [[exp(T), 0], [0, exp(T)^T]]  (bf16, partitions 0-63)
    W64 = persist.tile([64, 64], bf16)
    nc.vector.memset(W64[:], 0.0)
    nc.scalar.activation(W64[0:32, 0:32], ttab[:], AF.Exp)
    # exp(T)^T: transpose at partition 0 (transpose outputs must land at
    # PSUM partition 0), exp there, then DMA to partitions 32-63
    tps2 = sh_ps.tile([32, 32], f32, tag="sp")
    nc.tensor.transpose(tps2[:], ttab[:], id32[:])
    exTT0 = persist.tile([32, 32], bf16)
    nc.scalar.activation(exTT0[:], tps2[:], AF.Exp)
    nc.sync.dma_start(W64[32:64, 32:64], exTT0[:])
    exTT32 = persist.tile([64, 32], bf16)
    nc.sync.dma_start(exTT32[32:64, :], exTT0[:])

    sraw = persist.tile([64, 1], f32)
    nc.sync.dma_start(sraw[0:32, :], Sd)
    nc.sync.dma_start(sraw[32:64, :], Ed)
    expSE = persist.tile([64, 1], f32)  # exp(start) rows 0-31, exp(end) 32-63
    nc.scalar.activation(expSE[:], sraw[:], AF.Exp)

    # sum-selector for renorm: col0 sums rows 0-31 (u), col1 rows 32-63 (y)
    ones2 = persist.tile([64, 2], bf16)
    nc.vector.memset(ones2[:], 0.0)
    nc.vector.memset(ones2[0:32, 0:1], 1.0)
    nc.vector.memset(ones2[32:64, 1:2], 1.0)
    # broadcast selector: row0 -> partitions 0-31, row1 -> partitions 32-63
    # (built via iota+compare; sub-32-aligned partition writes are illegal)
    itc = persist.tile([2, 64], i16)
    nc.gpsimd.iota(itc[:], pattern=[[1, 64]], base=0, channel_multiplier=0)
    itcs = persist.tile([2, 64], i16)
    nc.vector.tensor_scalar(itcs[:], itc[:], 5, None, op0=ALU.logical_shift_right)
    itcsf = persist.tile([2, 64], f32)
    nc.vector.tensor_copy(itcsf[:], itcs[:])
    itp = persist.tile([2, 1], i16)
    nc.gpsimd.iota(itp[:], pattern=[[0, 1]], base=0, channel_multiplier=1)
    itpf = persist.tile([2, 1], f32)
    nc.vector.tensor_copy(itpf[:], itp[:])
    SEL2T = persist.tile([2, 64], bf16)
    nc.vector.tensor_scalar(SEL2T[:], itcsf[:], itpf[:], None, op0=ALU.is_equal)
    ones2c = persist.tile([2, 1], f32)
    nc.vector.memset(ones2c[:], 1.0)
    ones32 = persist.tile([32, 1], f32)
    nc.vector.memset(ones32[:], 1.0)

    # ---------------- numerator setup -------
    # Emissions are one-hot selected on the DVE straight from the scan's
    # chunk tiles (partitions 64-127), so Y lives there too.
    NQ = T // 16  # 48
    Yi = persist.tile([128, T], i32)
    nc.sync.dma_start(Yi[64:128, :], Yd)
    Yfb = persist.tile([128, T], bf16)
    nc.vector.tensor_copy(Yfb[64:128, :], Yi[64:128, :])
    iotaJ = persist.tile([128, 32 * 32], i16)
    nc.gpsimd.iota(iotaJ[64:128, :], pattern=[[0, 32], [1, 32]], base=0,
                   channel_multiplier=0)
    iotaJb = persist.tile([128, 32 * 32], bf16)
    nc.vector.tensor_copy(iotaJb[64:128, :], iotaJ[64:128, :])
    emacc = persist.tile([128, 24], f32)

    # flat Y (partitions 0-63) for the transition pair indices
    Yf_ = persist.tile([64, T], f32)
    nc.sync.dma_start(Yi[0:64, :], Yd)
    nc.vector.tensor_copy(Yf_[:], Yi[0:64, :])
    NP = 800
    pidx = persist.tile([64, NP], f32)
    nc.vector.scalar_tensor_tensor(pidx[:, 0:767], Yf_[:, 0:767], 32.0,
                                   Yf_[:, 1:768], op0=ALU.mult, op1=ALU.add)
    nc.vector.tensor_scalar_add(pidx[:, 767:768], Yf_[:, 0:1], 1024.0)
    nc.vector.tensor_scalar_add(pidx[:, 768:769], Yf_[:, 767:768], 1056.0)
    nc.vector.memset(pidx[:, 769:800], 1088.0)
    pidx16 = persist.tile([64, NP], i16)
    nc.vector.tensor_copy(pidx16[:], pidx[:])
    dpool = es.enter_context(tc.tile_pool(name="dram", bufs=1, space="DRAM"))
    pd = dpool.tile([64, NP], i16)
    pdw = nc.sync.dma_start(pd[:], pidx16[:])
    NPC = NP // 16  # 50
    PIDX = persist.tile([128, 8 * NPC], i16)

    # PIDX wrap DMAs (one per batch, ~50 small packets each) drained a
    # few per scan step on the sync queue
    sync_q = []
    for b in range(B):
        g, tau = b % 8, b // 8

        def _pidxw(b=b, g=g, tau=tau):
            wi = nc.sync.dma_start(
                PIDX[16 * g:16 * g + 16, NPC * tau:NPC * tau + NPC],
                pd[b:b + 1, :].rearrange("a (c p) -> a p c", p=16),
            )
            add_dep_helper(wi.ins, pdw.ins, sync=True,
                           reason="wrap read waits for dram roundtrip write")
        sync_q.append(_pidxw)

    # table: [T flat 1024 | start 32 | end 32 | zeros 4] on 128 partitions
    TTAB = persist.tile([128, 1092], f32)
    nc.gpsimd.memset(TTAB[0:1, :], 0.0)
    nc.sync.dma_start(TTAB[0:1, 0:1024], Td.rearrange("i j -> (i j)"))
    nc.sync.dma_start(TTAB[0:1, 1024:1056], Sd)
    nc.sync.dma_start(TTAB[0:1, 1056:1088], Ed)
    nc.gpsimd.partition_broadcast(TTAB[:], TTAB[0:1, :])

    # selection matrix for the per-group transition combine matmuls
    iop = persist.tile([128, 1], i16)
    nc.gpsimd.iota(iop[:], pattern=[[0, 1]], base=0, channel_multiplier=1)
    iog = persist.tile([128, 8], i16)
    nc.gpsimd.iota(iog[:], pattern=[[1, 8]], base=0, channel_multiplier=0)
    iogf = persist.tile([128, 8], f32)
    nc.vector.tensor_copy(iogf[:], iog[:])
    pdiv = persist.tile([128, 1], i16)
    nc.vector.tensor_scalar(pdiv[:], iop[:], 4, None, op0=ALU.logical_shift_right)
    pdivf = persist.tile([128, 1], f32)
    nc.vector.tensor_copy(pdivf[:], pdiv[:])
    SELe = persist.tile([128, 8], f32)
    nc.vector.tensor_scalar(SELe[:], iogf[:], pdivf[:], None, op0=ALU.is_equal)
    SELt = persist.tile([128, 8], f32)
    nc.vector.tensor_scalar_mul(SELt[:], SELe[:], 1.0 / 16.0)

    tpart = persist.tile([128, 8], f32)

    # ---------------- emission pipeline ----------------
    # paired bf16 exp(X) chunk staging: pair r cols [64r',64r'+32) =
    # exp(x_r), [64r'+32,64r'+64) = exp(x_{767-r}) (r' chunk-local);
    # partitions 64-127 (batch)
    CHT = {}  # chunk i -> staged paired bf16 tile

    XCH = {}  # chunk i -> (fwd tile, bwd tile, lo, blo) for the emission pass

    def chunk_prep_head(i):
        # DMA + exp-cast for chunk i (pairs 32i..32i+31)
        lo = CH * i
        xp = xinb.tile([128, CH * 64], bf16)
        xpv = xp[64:128, :].rearrange("p (r h j) -> p r h j", h=2, j=32)
        CHT[i] = xp
        xf = xin.tile([128, CH * 32], f32)
        nc.sync.dma_start(xf[64:128, :], Xf[:, 32 * lo:32 * (lo + CH)])
        nc.scalar.activation(
            xpv[:, :, 0, :],
            xf[64:128, :].rearrange("p (t j) -> p t j", j=32), AF.Exp)
        xb = xin.tile([128, CH * 32], f32)
        blo = T - lo - CH  # covers t' = blo .. blo+CH-1 (= 767-r desc)
        nc.sync.dma_start(xb[64:128, :], Xf[:, 32 * blo:32 * (blo + CH)])
        # reversed read: pair r = 767-t' ascends as t' descends
        nc.scalar.activation(
            xpv[:, :, 1, :],
            xb[64:128, :].rearrange("p (t j) -> p t j", j=32)[:, ::-1, :],
            AF.Exp)
        XCH[i] = (xf, xb, lo, blo)

    # transposes land in 16-pair PSUM block tiles; one ACT copy per full
    # block moves it to the SBUF EX buffer (so scan muls carry at most one
    # extra semaphore wait per 16 steps)
    EX = persist.tile([64, 64 * NPAIR], bf16)
    BLK = 16
    TPB = {}  # block index -> PSUM block tile

    def emit_pair(r):
        b, k = r // BLK, r % BLK
        if k == 0:
            TPB[b] = tp_ps.tile([64, 64 * BLK], bf16, tag="tp", name=f"tpb_{b}")
        xp = CHT[r // CH]
        rl = r % CH
        nc.tensor.transpose(TPB[b][:, 64 * k:64 * k + 64],
                            xp[64:128, 64 * rl:64 * rl + 64],
                            ident[64:128, :])
        if k == BLK - 1:
            nc.scalar.copy(EX[:, 64 * BLK * b:64 * BLK * (b + 1)], TPB[b][:])
            del TPB[b]

    # ---------------- numerator gathers (merged, all on gpsimd) -------
    # one shared output buffer for both merged gathers (sequential on the
    # gpsimd queue, WAR-serialized by tile deps)
    gtile = persist.tile([128, 8 * NP], f32)

    # transition gather on gpsimd; accums on DVE issued much later (data
    # certainly ready, so they fill scan idle instead of blocking it)

    def gather_trans():
        nc.gpsimd.ap_gather(gtile[:], TTAB[:], PIDX[:],
                            channels=128, num_elems=1092, d=1, num_idxs=8 * NP)

    def accum_tau(tau):
        sl = gtile[:, NP * tau:NP * tau + NP]
        nc.vector.tensor_scalar(sl, sl, 1.0, 0.0, op0=ALU.mult,
                                op1=ALU.add,
                                accum_out=tpart[:, tau:tau + 1])

    # one-hot emission select+accumulate straight from a scan chunk tile:
    # OH = (j == Y[b,t]), then accumulate sum_t X[b,t,Y[b,t]] per batch
    ohscr = persist.tile([128, CH * 32], bf16)

    def emis_chunk(xtile, tlo, col):
        ysl = Yfb[64:128, tlo:tlo + CH]
        yrep = ysl.unsqueeze(2).broadcast_to((64, CH, 32))
        ioj = iotaJb[64:128, :].rearrange("p (t j) -> p t j", j=32)
        oh = ohscr[64:128, :].rearrange("p (t j) -> p t j", j=32)
        nc.vector.tensor_tensor(oh, ioj, yrep, op=ALU.is_equal)
        nc.vector.scalar_tensor_tensor(
            ohscr[64:128, :], ohscr[64:128, :], 1.0, xtile[64:128, :],
            op0=ALU.bypass, op1=ALU.mult,
            accum_out=emacc[64:128, col:col + 1])

    # ---------------- the scan ----------------
    # +1 slot: a final renorm of the last state keeps the combine's
    # chain-product inside the scalar engine's Ln range
    NREN = len(range(RENORM, NPAIR - REN_LAG, RENORM)) + 1
    rst = persist.tile([2, 64 * NREN], bf16)
    ren_slot = [0]

    U2 = [persist.tile([64, 64], bf16, name=f"u2_{k}") for k in range(3)]

    def st(r):
        return U2[r % 3]

    def renorm_a(r):
        # measure sums of both chains on state r.  bf16 scales so the
        # applied factor and the Ln-logged factor are bit-identical.
        sp = sh_ps.tile([2, 64], f32, tag="sp")
        nc.tensor.matmul(sp[:], ones2[:], st(r)[:], tile_position=(0, 0))
        srec = rscr.tile([2, 64], f32)
        nc.vector.reciprocal_approx_fast(srec[:], sp[:])
        m = ren_slot[0]
        ren_slot[0] += 1
        rsl = rst[:, 64 * m:64 * m + 64]
        nc.vector.tensor_copy(rsl, srec[:])
        return rsl

    def renorm_b(rsl):
        rb = sh_ps.tile([64, 64], f32, tag="rb")
        nc.tensor.matmul(rb[:], SEL2T[:], rsl, tile_position=(0, 0))
        return rb

    # prologue: chunk 0 prepped with the first transposes, chunk 1 queued
    chunk_prep_head(0)
    for rr in range(TLEAD + 1):
        emit_pair(rr)
    chunk_prep_head(1)

    # u_0 = exp(start) * ex_0 ; y_767 = exp(end) * ex_767
    nc.vector.tensor_scalar_mul(st(0)[:], EX[:, 0:64], expSE[:, 0:1])

    pend_a = None   # rsl awaiting broadcast
    pend_b = {}     # r -> rb PSUM tile to fuse at step r
    for r in range(1, NPAIR):
        i = r // CH
        if r % CH == 0 and i + 1 < NCHUNK:
            chunk_prep_head(i + 1)
        if r % 8 == 5 and sync_q:
            for _ in range(4):
                if sync_q:
                    sync_q.pop(0)()
        if r == 150:
            gather_trans()
        if r >= 280 and (r - 280) % 2 == 0 and (r - 280) // 2 < 8:
            accum_tau((r - 280) // 2)
        if r % CH == 16 and r // CH in XCH:
            xf_, xb_, lo_, blo_ = XCH[r // CH]
            emis_chunk(xf_, lo_, 2 * (r // CH))
        if r % CH == 18 and r // CH in XCH:
            xf_, xb_, lo_, blo_ = XCH.pop(r // CH)
            emis_chunk(xb_, blo_, 2 * (r // CH) + 1)
        if r + TLEAD < NPAIR:
            emit_pair(r + TLEAD)
        vp = sc_ps.tile([64, 64], f32, tag="sc")
        nc.tensor.matmul(vp[:], W64[:], st(r - 1)[:], tile_position=(0, 0))
        exs = EX[:, 64 * r:64 * r + 64]
        rb = pend_b.pop(r, None)
        if rb is None:
            nc.vector.tensor_mul(st(r)[:], vp[:], exs)
        else:
            u2t = rscr.tile([64, 64], f32)
            nc.vector.tensor_mul(u2t[:], vp[:], exs)
            nc.vector.tensor_mul(st(r)[:], u2t[:], rb[:])
        if pend_a is not None:
            pend_b[r + REN_LAG - 1] = renorm_b(pend_a)
            pend_a = None
        if r % RENORM == 0 and r + REN_LAG < NPAIR and ren_slot[0] < NREN - 1:
            pend_a = renorm_a(r)

    # ---------------- combine: Z = u_383^T exp(T) y_384 ----------------
    # final renorm: both chains scaled to unit sum (and logged) so the
    # product stays well inside the Ln table range
    rslF = renorm_a(NPAIR - 1)
    rbF = renorm_b(rslF)
    last = persist.tile([64, 64], bf16)
    nc.vector.tensor_mul(last[:], st(NPAIR - 1)[:], rbF[:])
    w383 = sc_ps.tile([32, 64], f32, tag="sc")
    nc.tensor.matmul(w383[:], exTT32[32:64, :], last[32:64, :],
                     tile_position=(32, 0))
    q = persist.tile([32, 64], f32)
    nc.vector.tensor_mul(q[:], w383[:], last[0:32, :])
    combo = sh_ps.tile([1, 64], f32, tag="sp")
    nc.tensor.matmul(combo[:], ones32[:], q[:], tile_position=(0, 0))

    # numerator: emission chunk sums reduced per batch then transposed to
    # [1,64] via the PE; transition group sums folded via SELt matmuls
    if 11 in XCH:
        xf11, xb11, lo11, blo11 = XCH.pop(11)
        emis_chunk(xf11, lo11, 22)
        emis_chunk(xb11, blo11, 23)
    emsum = persist.tile([128, 1], f32)
    nc.vector.tensor_reduce(emsum[64:128, :], emacc[64:128, :], AX.X, ALU.add)
    emT = sc_ps.tile([1, 64], f32, tag="sc")
    nc.tensor.transpose(emT[:], emsum[64:128, :], identF[64:128, :])
    nump = sh_ps.tile([1, 64], f32, tag="rb")
    for tau in range(8):
        sl = nump[0:1, 8 * tau:8 * tau + 8]
        nc.tensor.matmul(sl, tpart[:, tau:tau + 1], SELt[:], start=True,
                         stop=True, tile_position=(0, 0))

    # ---------------- final assembly ----------------
    lncombo = persist.tile([1, 64], f32)
    nc.scalar.activation(lncombo[:], combo[:], AF.Ln)
    lnr = persist.tile([2, 64 * NREN], f32)
    nc.scalar.activation(lnr[:], rst[:], AF.Ln)
    lnrsum = persist.tile([2, 64], f32)
    nc.vector.tensor_reduce(lnrsum[:], lnr[:].rearrange("p (m b) -> p b m", b=64),
                            AX.X, ALU.add)
    lnboth = sh_ps.tile([1, 64], f32, tag="sp")
    nc.tensor.matmul(lnboth[:], ones2c[:], lnrsum[:], tile_position=(0, 0))
    f1 = persist.tile([1, 64], f32)
    nc.vector.tensor_sub(f1[:], nump[:], lncombo[:])
    f1b = persist.tile([1, 64], f32)
    nc.vector.tensor_add(f1b[:], f1[:], emT[:])
    f2 = persist.tile([1, 64], f32)
    nc.vector.tensor_add(f2[:], f1b[:], lnboth[:])
    nc.sync.dma_start(Od, f2[:])

    if _DEBUG:
        def dout(name, ap):
            d = nc.dram_tensor(name, list(ap.shape), ap.dtype,
                               kind="ExternalOutput").ap()
            nc.sync.dma_start(d, ap)
        dout("d_u2", last[:]); dout("d_rst", rst[:]); dout("d_q", q[:])
        dout("d_empart", empart[:]); dout("d_tpart", tpart[:])
        dout("d_f1", f1[:]); dout("d_lnrsum", lnrsum[:])

    es.close()


def _build():
    import concourse.tile as tile
    from concourse import bacc, mybir

    f32 = mybir.dt.float32
    i32 = mybir.dt.int32

    nc = bacc.Bacc("TRN2", target_bir_lowering=False, debug=False,
                   enable_asserts=False, num_devices=NCORES)
    Xd = nc.dram_tensor("x", [B, T, NTAG], f32, kind="ExternalInput").ap()
    Yd = nc.dram_tensor("y", [B, T], i32, kind="ExternalInput").ap()
    Td = nc.dram_tensor("t", [NTAG, NTAG], f32, kind="ExternalInput").ap()
    Sd = nc.dram_tensor("s", [NTAG], f32, kind="ExternalInput").ap()
    Ed = nc.dram_tensor("e", [NTAG], f32, kind="ExternalInput").ap()
    Od = nc.dram_tensor("o", [B], f32, kind="ExternalOutput").ap()
    with tile.TileContext(nc) as tc:
        _emit(tc, nc, (Xd, Yd, Td, Sd, Ed, Od))
    nc.compile()
    return nc


def _numpy_fallback(X, Y, mask, transition, start_trans, end_trans):
    X = np.asarray(X, np.float64)
    Y = np.asarray(Y, np.int64)
    m = np.asarray(mask, bool)
    Tm = np.asarray(transition, np.float64)
    st = np.asarray(start_trans, np.float64)
    en = np.asarray(end_trans, np.float64)
    bs, sl, nt = X.shape
    rb = np.arange(bs)
    mf = m.astype(np.float64)
    score = st[Y[:, 0]] + X[rb, 0, Y[:, 0]]
    emit = np.take_along_axis(X[:, 1:], Y[:, 1:, None], axis=2)[..., 0]
    tr = Tm[Y[:, :-1], Y[:, 1:]]
    score = score + np.sum((tr + emit) * mf[:, 1:], axis=1)
    each_len = m.sum(1).astype(np.int64)
    last_tag = Y[rb, each_len - 1]
    score = score + en[last_tag] * mf[rb, each_len - 1]
    alpha = st[None, :] + X[:, 0]
    for t in range(1, sl):
        s = alpha[:, :, None] + Tm[None] + X[:, t][:, None, :]
        mx = s.max(1)
        new = mx + np.log(np.exp(s - mx[:, None, :]).sum(1))
        alpha = np.where(m[:, t][:, None], new, alpha)
    mx = (alpha + en).max(1)
    logZ = mx + np.log(np.exp(alpha + en - mx[:, None]).sum(1))
    return (score - logZ).astype(np.float32)


def kernel(X, Y, mask, transition, start_trans, end_trans):
    X = np.ascontiguousarray(np.asarray(X, dtype=np.float32))
    Yc = np.ascontiguousarray(np.asarray(Y).astype(np.int32))
    Tm = np.ascontiguousarray(np.asarray(transition, dtype=np.float32))
    st = np.ascontiguousarray(np.asarray(start_trans, dtype=np.float32))
    en = np.ascontiguousarray(np.asarray(end_trans, dtype=np.float32))
    mk = np.asarray(mask)

    if X.shape != (BS, T, NTAG) or not bool(mk.all()):
        return _numpy_fallback(X, Y, mask, transition, start_trans, end_trans)

    from concourse import bass_utils

    if "nc" not in _state:
        _state["nc"] = _build()
    nc = _state["nc"]

    in_maps = []
    for c in range(NCORES):
        sl = slice(B * c, B * (c + 1))
        in_maps.append({"x": X[sl], "y": Yc[sl], "t": Tm, "s": st, "e": en})
    res = bass_utils.run_bass_kernel_spmd(nc, in_maps, core_ids=list(range(NCORES)))
    out = np.concatenate([res.results[c]["o"] for c in range(NCORES)])
    return out.astype(np.float32)


if __name__ == "__main__":
    sys.path.insert(0, "/root/problem")
    import reference

    inputs = reference.setup_inputs()
    inputs = {k: np.asarray(v) for k, v in inputs.items()}
    exp = np.asarray(reference.reference(**inputs))
    act = kernel(**inputs)
    err = np.abs(act - exp) / np.maximum(np.abs(exp), 1e-6)
    print("max rel err:", err.max(), "mean:", err.mean())


# revision 45
# speedup vs baseline: 1.3855x; 1.0208x over previous
"""CRF loss (log-likelihood) kernel for Trainium2, 8 NeuronCores.

Strategy (v3):
  - Data-parallel: batch 512 sharded as 64 per core.
  - Denominator: exp-space forward+backward scans MERGED into one serial
    chain of 64x64 bf16 matmuls (block-diagonal weights [[exp(T),0],
    [0,exp(T)^T]]) + one DVE multiply per step; chains meet in the middle
    (384 steps).  The per-step DVE multiply reads BOTH operands from PSUM
    (the scan matmul output and the just-in-time pair transpose output),
    so it carries a single inline semaphore wait and no SBUF emission
    buffer is needed.
  - Emissions: X is DMA'd in fp32 chunks, exp-cast to bf16 by the scalar
    engine into a paired layout (pair r = [x_r | x_{767-r}], the bwd half
    time-reversed via negative-stride reads), then PE-transposed to PSUM
    2 steps ahead of consumption.
  - Renormalization every 8 steps: ones-matmul chain sums -> DVE
    reciprocal (logged in bf16 so applied == logged) -> PE outer-product
    broadcast -> one extra DVE multiply fused into the chain 4 steps
    later.
  - Numerator: ONE merged gpsimd ap_gather for emissions (8 batch-groups
    x 48 wrapped columns against a 12288-element per-partition table) and
    ONE for transitions, with all masking/reduction post-ops on gpsimd so
    the DVE scan stream is never blocked.  Wrapped-layout DMAs (64+64+64
    one-per-batch descriptors) are drained a few per scan step.
"""

import os
import sys

import numpy as np

for _p in ("/opt/trn_rl_repo", "/root/.axon_site/_ro/trn_rl_repo"):
    if os.path.isdir(_p) and _p not in sys.path:
        sys.path.insert(0, _p)

BS, T, NTAG = 512, 768, 32
NCORES = 8
B = BS // NCORES  # 64 batch per core
NPAIR = T // 2  # 384 merged scan steps
CH = 32         # pairs per pipeline chunk
NCHUNK = NPAIR // CH  # 12
RENORM = 8
REN_LAG = 4     # renorm measured at r, scale fused into step r+REN_LAG
TLEAD = 20      # transpose emitted TLEAD steps ahead of consumption

_state = {}
_DEBUG = False


def _emit(tc, nc, aps):
    import concourse.bass as bass
    from concourse import masks, mybir
    from concourse.tile import add_dep_helper

    f32 = mybir.dt.float32
    bf16 = mybir.dt.bfloat16
    i32 = mybir.dt.int32
    i16 = mybir.dt.int16
    AF = mybir.ActivationFunctionType
    ALU = mybir.AluOpType
    AX = mybir.AxisListType

    Xd, Yd, Td, Sd, Ed, Od = aps
    Xf = Xd.rearrange("b t j -> b (t j)")  # [64, 24576]

    from contextlib import ExitStack

    es = _state["es"] = ExitStack()
    persist = es.enter_context(tc.tile_pool(name="persist", bufs=1))
    xin = es.enter_context(tc.tile_pool(name="xin", bufs=3))
    xinb = es.enter_context(tc.tile_pool(name="xinb", bufs=3))
    sc_ps = es.enter_context(tc.tile_pool(name="sc_ps", bufs=3, space="PSUM"))
    tp_ps = es.enter_context(tc.tile_pool(name="tp_ps", bufs=3, space="PSUM"))
    sh_ps = es.enter_context(tc.tile_pool(name="sh_ps", bufs=1, space="PSUM"))
    rscr = es.enter_context(tc.tile_pool(name="rscr", bufs=2))

    # ---------------- constants ----------------
    ident = persist.tile([128, 64], bf16)   # identity at partitions 64-127
    masks.make_identity(nc, ident[64:128, :])
    identF = persist.tile([128, 64], f32)   # f32 identity for fp32 transposes
    masks.make_identity(nc, identF[64:128, :])
    id32 = persist.tile([32, 32], f32)
    masks.make_identity(nc, id32[:])

    ttab = persist.tile([32, 32], f32)
    nc.sync.dma_start(ttab[:], Td)
    # W64 = [[exp(T), 0], [0, exp(T)^T]]  (bf16, partitions 0-63)
    W64 = persist.tile([64, 64], bf16)
    nc.vector.memset(W64[:], 0.0)
    nc.scalar.activation(W64[0:32, 0:32], ttab[:], AF.Exp)
    # exp(T)^T: transpose at partition 0 (transpose outputs must land at
    # PSUM partition 0), exp there, then DMA to partitions 32-63
    tps2 = sh_ps.tile([32, 32], f32, tag="sp")
    nc.tensor.transpose(tps2[:], ttab[:], id32[:])
    exTT0 = persist.tile([32, 32], bf16)
    nc.scalar.activation(exTT0[:], tps2[:], AF.Exp)
    nc.sync.dma_start(W64[32:64, 32:64], exTT0[:])
    exTT32 = persist.tile([64, 32], bf16)
    nc.sync.dma_start(exTT32[32:64, :], exTT0[:])

    sraw = persist.tile([64, 1], f32)
    nc.sync.dma_start(sraw[0:32, :], Sd)
    nc.sync.dma_start(sraw[32:64, :], Ed)
    expSE = persist.tile([64, 1], f32)  # exp(start) rows 0-31, exp(end) 32-63
    nc.scalar.activation(expSE[:], sraw[:], AF.Exp)

    # sum-selector for renorm: col0 sums rows 0-31 (u), col1 rows 32-63 (y)
    ones2 = persist.tile([64, 2], bf16)
    nc.vector.memset(ones2[:], 0.0)
    nc.vector.memset(ones2[0:32, 0:1], 1.0)
    nc.vector.memset(ones2[32:64, 1:2], 1.0)
    # broadcast selector: row0 -> partitions 0-31, row1 -> partitions 32-63
    # (built via iota+compare; sub-32-aligned partition writes are illegal)
    itc = persist.tile([2, 64], i16)
    nc.gpsimd.iota(itc[:], pattern=[[1, 64]], base=0, channel_multiplier=0)
    itcs = persist.tile([2, 64], i16)
    nc.vector.tensor_scalar(itcs[:], itc[:], 5, None, op0=ALU.logical_shift_right)
    itcsf = persist.tile([2, 64], f32)
    nc.vector.tensor_copy(itcsf[:], itcs[:])
    itp = persist.tile([2, 1], i16)
    nc.gpsimd.iota(itp[:], pattern=[[0, 1]], base=0, channel_multiplier=1)
    itpf = persist.tile([2, 1], f32)
    nc.vector.tensor_copy(itpf[:], itp[:])
    SEL2T = persist.tile([2, 64], bf16)
    nc.vector.tensor_scalar(SEL2T[:], itcsf[:], itpf[:], None, op0=ALU.is_equal)
    ones2c = persist.tile([2, 1], f32)
    nc.vector.memset(ones2c[:], 1.0)
    ones32 = persist.tile([32, 1], f32)
    nc.vector.memset(ones32[:], 1.0)

    # ---------------- numerator setup -------
    # Emissions are one-hot selected on the DVE straight from the scan's
    # chunk tiles (partitions 64-127), so Y lives there too.
    NQ = T // 16  # 48
    Yi = persist.tile([128, T], i32)
    nc.sync.dma_start(Yi[64:128, :], Yd)
    Yfb = persist.tile([128, T], bf16)
    nc.vector.tensor_copy(Yfb[64:128, :], Yi[64:128, :])
    iotaJ = persist.tile([128, 32 * 32], i16)
    nc.gpsimd.iota(iotaJ[64:128, :], pattern=[[0, 32], [1, 32]], base=0,
                   channel_multiplier=0)
    iotaJb = persist.tile([128, 32 * 32], bf16)
    nc.vector.tensor_copy(iotaJb[64:128, :], iotaJ[64:128, :])
    emacc = persist.tile([128, 24], f32)

    # flat Y (partitions 0-63) for the transition pair indices
    Yf_ = persist.tile([64, T], f32)
    nc.sync.dma_start(Yi[0:64, :], Yd)
    nc.vector.tensor_copy(Yf_[:], Yi[0:64, :])
    NP = 800
    pidx = persist.tile([64, NP], f32)
    nc.vector.scalar_tensor_tensor(pidx[:, 0:767], Yf_[:, 0:767], 32.0,
                                   Yf_[:, 1:768], op0=ALU.mult, op1=ALU.add)
    nc.vector.tensor_scalar_add(pidx[:, 767:768], Yf_[:, 0:1], 1024.0)
    nc.vector.tensor_scalar_add(pidx[:, 768:769], Yf_[:, 767:768], 1056.0)
    nc.vector.memset(pidx[:, 769:800], 1088.0)
    pidx16 = persist.tile([64, NP], i16)
    nc.vector.tensor_copy(pidx16[:], pidx[:])
    dpool = es.enter_context(tc.tile_pool(name="dram", bufs=1, space="DRAM"))
    pd = dpool.tile([64, NP], i16)
    pdw = nc.sync.dma_start(pd[:], pidx16[:])
    NPC = NP // 16  # 50
    PIDX = persist.tile([128, 8 * NPC], i16)

    # PIDX wrap DMAs (one per batch, ~50 small packets each) drained a
    # few per scan step on the sync queue
    sync_q = []
    for b in range(B):
        g, tau = b % 8, b // 8

        def _pidxw(b=b, g=g, tau=tau):
            wi = nc.sync.dma_start(
                PIDX[16 * g:16 * g + 16, NPC * tau:NPC * tau + NPC],
                pd[b:b + 1, :].rearrange("a (c p) -> a p c", p=16),
            )
            add_dep_helper(wi.ins, pdw.ins, sync=True,
                           reason="wrap read waits for dram roundtrip write")
        sync_q.append(_pidxw)

    # table: [T flat 1024 | start 32 | end 32 | zeros 4] on 128 partitions
    TTAB = persist.tile([128, 1092], f32)
    nc.gpsimd.memset(TTAB[0:1, :], 0.0)
    nc.sync.dma_start(TTAB[0:1, 0:1024], Td.rearrange("i j -> (i j)"))
    nc.sync.dma_start(TTAB[0:1, 1024:1056], Sd)
    nc.sync.dma_start(TTAB[0:1, 1056:1088], Ed)
    nc.gpsimd.partition_broadcast(TTAB[:], TTAB[0:1, :])
    # dummy gather: loads the gpsimd gather library NOW, so its global
    # engine-drain barrier fires during setup instead of mid-scan
    didx = persist.tile([128, 4], i16)
    nc.vector.memset(didx[:], 0)
    dout_g = persist.tile([128, 64], f32)
    nc.gpsimd.ap_gather(dout_g[:], TTAB[:], didx[:],
                        channels=128, num_elems=1092, d=1, num_idxs=64)

    # selection matrix for the per-group transition combine matmuls
    iop = persist.tile([128, 1], i16)
    nc.gpsimd.iota(iop[:], pattern=[[0, 1]], base=0, channel_multiplier=1)
    iog = persist.tile([128, 8], i16)
    nc.gpsimd.iota(iog[:], pattern=[[1, 8]], base=0, channel_multiplier=0)
    iogf = persist.tile([128, 8], f32)
    nc.vector.tensor_copy(iogf[:], iog[:])
    pdiv = persist.tile([128, 1], i16)
    nc.vector.tensor_scalar(pdiv[:], iop[:], 4, None, op0=ALU.logical_shift_right)
    pdivf = persist.tile([128, 1], f32)
    nc.vector.tensor_copy(pdivf[:], pdiv[:])
    SELe = persist.tile([128, 8], f32)
    nc.vector.tensor_scalar(SELe[:], iogf[:], pdivf[:], None, op0=ALU.is_equal)
    SELt = persist.tile([128, 8], f32)
    nc.vector.tensor_scalar_mul(SELt[:], SELe[:], 1.0 / 16.0)

    tpart = persist.tile([128, 8], f32)

    # ---------------- emission pipeline ----------------
    # paired bf16 exp(X) chunk staging: pair r cols [64r',64r'+32) =
    # exp(x_r), [64r'+32,64r'+64) = exp(x_{767-r}) (r' chunk-local);
    # partitions 64-127 (batch)
    CHT = {}  # chunk i -> staged paired bf16 tile

    XCH = {}  # chunk i -> (fwd tile, bwd tile, lo, blo) for the emission pass

    def chunk_prep_head(i):
        # DMA + exp-cast for chunk i (pairs 32i..32i+31)
        lo = CH * i
        xp = xinb.tile([128, CH * 64], bf16)
        xpv = xp[64:128, :].rearrange("p (r h j) -> p r h j", h=2, j=32)
        CHT[i] = xp
        xf = xin.tile([128, CH * 32], f32)
        nc.sync.dma_start(xf[64:128, :], Xf[:, 32 * lo:32 * (lo + CH)])
        nc.scalar.activation(
            xpv[:, :, 0, :],
            xf[64:128, :].rearrange("p (t j) -> p t j", j=32), AF.Exp)
        xb = xin.tile([128, CH * 32], f32)
        blo = T - lo - CH  # covers t' = blo .. blo+CH-1 (= 767-r desc)
        nc.sync.dma_start(xb[64:128, :], Xf[:, 32 * blo:32 * (blo + CH)])
        # reversed read: pair r = 767-t' ascends as t' descends
        nc.scalar.activation(
            xpv[:, :, 1, :],
            xb[64:128, :].rearrange("p (t j) -> p t j", j=32)[:, ::-1, :],
            AF.Exp)
        XCH[i] = (xf, xb, lo, blo)

    # transposes land in 16-pair PSUM block tiles; one ACT copy per full
    # block moves it to the SBUF EX buffer (so scan muls carry at most one
    # extra semaphore wait per 16 steps)
    EX = persist.tile([64, 64 * NPAIR], bf16)
    BLK = 16
    TPB = {}  # block index -> PSUM block tile

    def emit_pair(r):
        b, k = r // BLK, r % BLK
        if k == 0:
            TPB[b] = tp_ps.tile([64, 64 * BLK], bf16, tag="tp", name=f"tpb_{b}")
        xp = CHT[r // CH]
        rl = r % CH
        nc.tensor.transpose(TPB[b][:, 64 * k:64 * k + 64],
                            xp[64:128, 64 * rl:64 * rl + 64],
                            ident[64:128, :])
        if k == BLK - 1:
            nc.scalar.copy(EX[:, 64 * BLK * b:64 * BLK * (b + 1)], TPB[b][:])
            del TPB[b]

    # ---------------- numerator gathers (merged, all on gpsimd) -------
    # one shared output buffer for both merged gathers (sequential on the
    # gpsimd queue, WAR-serialized by tile deps)
    gtile = persist.tile([128, 8 * NP], f32)

    # transition gather on gpsimd; accums on DVE issued much later (data
    # certainly ready, so they fill scan idle instead of blocking it)

    def gather_trans():
        nc.gpsimd.ap_gather(gtile[:], TTAB[:], PIDX[:],
                            channels=128, num_elems=1092, d=1, num_idxs=8 * NP)

    def accum_tau(tau):
        sl = gtile[:, NP * tau:NP * tau + NP]
        nc.vector.tensor_scalar(sl, sl, 1.0, 0.0, op0=ALU.mult,
                                op1=ALU.add,
                                accum_out=tpart[:, tau:tau + 1])

    # one-hot emission select+accumulate straight from a scan chunk tile:
    # OH = (j == Y[b,t]), then accumulate sum_t X[b,t,Y[b,t]] per batch
    ohscr = persist.tile([128, CH * 32], bf16)

    def emis_chunk(xtile, tlo, col):
        ysl = Yfb[64:128, tlo:tlo + CH]
        yrep = ysl.unsqueeze(2).broadcast_to((64, CH, 32))
        ioj = iotaJb[64:128, :].rearrange("p (t j) -> p t j", j=32)
        oh = ohscr[64:128, :].rearrange("p (t j) -> p t j", j=32)
        nc.vector.tensor_tensor(oh, ioj, yrep, op=ALU.is_equal)
        nc.vector.scalar_tensor_tensor(
            ohscr[64:128, :], ohscr[64:128, :], 1.0, xtile[64:128, :],
            op0=ALU.bypass, op1=ALU.mult,
            accum_out=emacc[64:128, col:col + 1])

    # ---------------- the scan ----------------
    # +1 slot: a final renorm of the last state keeps the combine's
    # chain-product inside the scalar engine's Ln range
    NREN = len(range(RENORM, NPAIR - REN_LAG, RENORM)) + 1
    rst = persist.tile([2, 64 * NREN], bf16)
    ren_slot = [0]

    U2 = [persist.tile([64, 64], bf16, name=f"u2_{k}") for k in range(3)]

    def st(r):
        return U2[r % 3]

    def renorm_a(r):
        # measure sums of both chains on state r.  bf16 scales so the
        # applied factor and the Ln-logged factor are bit-identical.
        sp = sh_ps.tile([2, 64], f32, tag="sp")
        nc.tensor.matmul(sp[:], ones2[:], st(r)[:], tile_position=(0, 0))
        srec = rscr.tile([2, 64], f32)
        nc.vector.reciprocal_approx_fast(srec[:], sp[:])
        m = ren_slot[0]
        ren_slot[0] += 1
        rsl = rst[:, 64 * m:64 * m + 64]
        nc.vector.tensor_copy(rsl, srec[:])
        return rsl

    def renorm_b(rsl):
        rb = sh_ps.tile([64, 64], f32, tag="rb")
        nc.tensor.matmul(rb[:], SEL2T[:], rsl, tile_position=(0, 0))
        return rb

    # prologue: chunk 0 prepped with the first transposes, chunk 1 queued
    chunk_prep_head(0)
    for rr in range(TLEAD + 1):
        emit_pair(rr)
    chunk_prep_head(1)

    # u_0 = exp(start) * ex_0 ; y_767 = exp(end) * ex_767
    nc.vector.tensor_scalar_mul(st(0)[:], EX[:, 0:64], expSE[:, 0:1])

    pend_a = None   # rsl awaiting broadcast
    pend_b = {}     # r -> rb PSUM tile to fuse at step r
    for r in range(1, NPAIR):
        i = r // CH
        if r % CH == 0 and i + 1 < NCHUNK:
            chunk_prep_head(i + 1)
        if r % 8 == 5 and sync_q:
            for _ in range(8):
                if sync_q:
                    sync_q.pop(0)()
        if r == 100:
            gather_trans()
        if r >= 280 and (r - 280) % 2 == 0 and (r - 280) // 2 < 8:
            accum_tau((r - 280) // 2)
        if r % CH == 16 and r // CH in XCH:
            xf_, xb_, lo_, blo_ = XCH[r // CH]
            emis_chunk(xf_, lo_, 2 * (r // CH))
        if r % CH == 18 and r // CH in XCH:
            xf_, xb_, lo_, blo_ = XCH.pop(r // CH)
            emis_chunk(xb_, blo_, 2 * (r // CH) + 1)
        if r + TLEAD < NPAIR:
            emit_pair(r + TLEAD)
        vp = sc_ps.tile([64, 64], f32, tag="sc")
        nc.tensor.matmul(vp[:], W64[:], st(r - 1)[:], tile_position=(0, 0))
        exs = EX[:, 64 * r:64 * r + 64]
        rb = pend_b.pop(r, None)
        if rb is None:
            nc.vector.tensor_mul(st(r)[:], vp[:], exs)
        else:
            u2t = rscr.tile([64, 64], f32)
            nc.vector.tensor_mul(u2t[:], vp[:], exs)
            nc.vector.tensor_mul(st(r)[:], u2t[:], rb[:])
        if pend_a is not None:
            pend_b[r + REN_LAG - 1] = renorm_b(pend_a)
            pend_a = None
        if r % RENORM == 0 and r + REN_LAG < NPAIR and ren_slot[0] < NREN - 1:
            pend_a = renorm_a(r)

    # ---------------- combine: Z = u_383^T exp(T) y_384 ----------------
    # final renorm: both chains scaled to unit sum (and logged) so the
    # product stays well inside the Ln table range
    rslF = renorm_a(NPAIR - 1)
    rbF = renorm_b(rslF)
    last = persist.tile([64, 64], bf16)
    nc.vector.tensor_mul(last[:], st(NPAIR - 1)[:], rbF[:])
    w383 = sc_ps.tile([32, 64], f32, tag="sc")
    nc.tensor.matmul(w383[:], exTT32[32:64, :], last[32:64, :],
                     tile_position=(32, 0))
    q = persist.tile([32, 64], f32)
    nc.vector.tensor_mul(q[:], w383[:], last[0:32, :])
    combo = sh_ps.tile([1, 64], f32, tag="sp")
    nc.tensor.matmul(combo[:], ones32[:], q[:], tile_position=(0, 0))

    # numerator: emission chunk sums reduced per batch then transposed to
    # [1,64] via the PE; transition group sums folded via SELt matmuls
    if 11 in XCH:
        xf11, xb11, lo11, blo11 = XCH.pop(11)
        emis_chunk(xf11, lo11, 22)
        emis_chunk(xb11, blo11, 23)
    emsum = persist.tile([128, 1], f32)
    nc.vector.tensor_reduce(emsum[64:128, :], emacc[64:128, :], AX.X, ALU.add)
    emT = sc_ps.tile([1, 64], f32, tag="sc")
    nc.tensor.transpose(emT[:], emsum[64:128, :], identF[64:128, :])
    nump = sh_ps.tile([1, 64], f32, tag="rb")
    for tau in range(8):
        sl = nump[0:1, 8 * tau:8 * tau + 8]
        nc.tensor.matmul(sl, tpart[:, tau:tau + 1], SELt[:], start=True,
                         stop=True, tile_position=(0, 0))

    # ---------------- final assembly ----------------
    lncombo = persist.tile([1, 64], f32)
    nc.scalar.activation(lncombo[:], combo[:], AF.Ln)
    lnr = persist.tile([2, 64 * NREN], f32)
    nc.scalar.activation(lnr[:], rst[:], AF.Ln)
    lnrsum = persist.tile([2, 64], f32)
    nc.vector.tensor_reduce(lnrsum[:], lnr[:].rearrange("p (m b) -> p b m", b=64),
                            AX.X, ALU.add)
    lnboth = sh_ps.tile([1, 64], f32, tag="sp")
    nc.tensor.matmul(lnboth[:], ones2c[:], lnrsum[:], tile_position=(0, 0))
    f1 = persist.tile([1, 64], f32)
    nc.vector.tensor_sub(f1[:], nump[:], lncombo[:])
    f1b = persist.tile([1, 64], f32)
    nc.vector.tensor_add(f1b[:], f1[:], emT[:])
    f2 = persist.tile([1, 64], f32)
    nc.vector.tensor_add(f2[:], f1b[:], lnboth[:])
    nc.sync.dma_start(Od, f2[:])

    if _DEBUG:
        def dout(name, ap):
            d = nc.dram_tensor(name, list(ap.shape), ap.dtype,
                               kind="ExternalOutput").ap()
            nc.sync.dma_start(d, ap)
        dout("d_u2", last[:]); dout("d_rst", rst[:]); dout("d_q", q[:])
        dout("d_empart", empart[:]); dout("d_tpart", tpart[:])
        dout("d_f1", f1[:]); dout("d_lnrsum", lnrsum[:])

    es.close()


def _build():
    import concourse.tile as tile
    from concourse import bacc, mybir

    f32 = mybir.dt.float32
    i32 = mybir.dt.int32

    nc = bacc.Bacc("TRN2", target_bir_lowering=False, debug=False,
                   enable_asserts=False, num_devices=NCORES)
    Xd = nc.dram_tensor("x", [B, T, NTAG], f32, kind="ExternalInput").ap()
    Yd = nc.dram_tensor("y", [B, T], i32, kind="ExternalInput").ap()
    Td = nc.dram_tensor("t", [NTAG, NTAG], f32, kind="ExternalInput").ap()
    Sd = nc.dram_tensor("s", [NTAG], f32, kind="ExternalInput").ap()
    Ed = nc.dram_tensor("e", [NTAG], f32, kind="ExternalInput").ap()
    Od = nc.dram_tensor("o", [B], f32, kind="ExternalOutput").ap()
    with tile.TileContext(nc) as tc:
        _emit(tc, nc, (Xd, Yd, Td, Sd, Ed, Od))
    nc.compile()
    return nc


def _numpy_fallback(X, Y, mask, transition, start_trans, end_trans):
    X = np.asarray(X, np.float64)
    Y = np.asarray(Y, np.int64)
    m = np.asarray(mask, bool)
    Tm = np.asarray(transition, np.float64)
    st = np.asarray(start_trans, np.float64)
    en = np.asarray(end_trans, np.float64)
    bs, sl, nt = X.shape
    rb = np.arange(bs)
    mf = m.astype(np.float64)
    score = st[Y[:, 0]] + X[rb, 0, Y[:, 0]]
    emit = np.take_along_axis(X[:, 1:], Y[:, 1:, None], axis=2)[..., 0]
    tr = Tm[Y[:, :-1], Y[:, 1:]]
    score = score + np.sum((tr + emit) * mf[:, 1:], axis=1)
    each_len = m.sum(1).astype(np.int64)
    last_tag = Y[rb, each_len - 1]
    score = score + en[last_tag] * mf[rb, each_len - 1]
    alpha = st[None, :] + X[:, 0]
    for t in range(1, sl):
        s = alpha[:, :, None] + Tm[None] + X[:, t][:, None, :]
        mx = s.max(1)
        new = mx + np.log(np.exp(s - mx[:, None, :]).sum(1))
        alpha = np.where(m[:, t][:, None], new, alpha)
    mx = (alpha + en).max(1)
    logZ = mx + np.log(np.exp(alpha + en - mx[:, None]).sum(1))
    return (score - logZ).astype(np.float32)


def kernel(X, Y, mask, transition, start_trans, end_trans):
    X = np.ascontiguousarray(np.asarray(X, dtype=np.float32))
    Yc = np.ascontiguousarray(np.asarray(Y).astype(np.int32))
    Tm = np.ascontiguousarray(np.asarray(transition, dtype=np.float32))
    st = np.ascontiguousarray(np.asarray(start_trans, dtype=np.float32))
    en = np.ascontiguousarray(np.asarray(end_trans, dtype=np.float32))
    mk = np.asarray(mask)

    if X.shape != (BS, T, NTAG) or not bool(mk.all()):
        return _numpy_fallback(X, Y, mask, transition, start_trans, end_trans)

    from concourse import bass_utils

    if "nc" not in _state:
        _state["nc"] = _build()
    nc = _state["nc"]

    in_maps = []
    for c in range(NCORES):
        sl = slice(B * c, B * (c + 1))
        in_maps.append({"x": X[sl], "y": Yc[sl], "t": Tm, "s": st, "e": en})
    res = bass_utils.run_bass_kernel_spmd(nc, in_maps, core_ids=list(range(NCORES)))
    out = np.concatenate([res.results[c]["o"] for c in range(NCORES)])
    return out.astype(np.float32)


if __name__ == "__main__":
    sys.path.insert(0, "/root/problem")
    import reference

    inputs = reference.setup_inputs()
    inputs = {k: np.asarray(v) for k, v in inputs.items()}
    exp = np.asarray(reference.reference(**inputs))
    act = kernel(**inputs)
    err = np.abs(act - exp) / np.maximum(np.abs(exp), 1e-6)
    print("max rel err:", err.max(), "mean:", err.mean())


# revision 46
# speedup vs baseline: 1.4497x; 1.0463x over previous
"""CRF loss (log-likelihood) kernel for Trainium2, 8 NeuronCores.

Strategy (v3):
  - Data-parallel: batch 512 sharded as 64 per core.
  - Denominator: exp-space forward+backward scans MERGED into one serial
    chain of 64x64 bf16 matmuls (block-diagonal weights [[exp(T),0],
    [0,exp(T)^T]]) + one DVE multiply per step; chains meet in the middle
    (384 steps).  The per-step DVE multiply reads BOTH operands from PSUM
    (the scan matmul output and the just-in-time pair transpose output),
    so it carries a single inline semaphore wait and no SBUF emission
    buffer is needed.
  - Emissions: X is DMA'd in fp32 chunks, exp-cast to bf16 by the scalar
    engine into a paired layout (pair r = [x_r | x_{767-r}], the bwd half
    time-reversed via negative-stride reads), then PE-transposed to PSUM
    2 steps ahead of consumption.
  - Renormalization every 8 steps: ones-matmul chain sums -> DVE
    reciprocal (logged in bf16 so applied == logged) -> PE outer-product
    broadcast -> one extra DVE multiply fused into the chain 4 steps
    later.
  - Numerator: ONE merged gpsimd ap_gather for emissions (8 batch-groups
    x 48 wrapped columns against a 12288-element per-partition table) and
    ONE for transitions, with all masking/reduction post-ops on gpsimd so
    the DVE scan stream is never blocked.  Wrapped-layout DMAs (64+64+64
    one-per-batch descriptors) are drained a few per scan step.
"""

import os
import sys

import numpy as np

for _p in ("/opt/trn_rl_repo", "/root/.axon_site/_ro/trn_rl_repo"):
    if os.path.isdir(_p) and _p not in sys.path:
        sys.path.insert(0, _p)

BS, T, NTAG = 512, 768, 32
NCORES = 8
B = BS // NCORES  # 64 batch per core
NPAIR = T // 2  # 384 merged scan steps
CH = 32         # pairs per pipeline chunk
NCHUNK = NPAIR // CH  # 12
RENORM = 8
REN_LAG = 4     # renorm measured at r, scale fused into step r+REN_LAG
TLEAD = 20      # transpose emitted TLEAD steps ahead of consumption

_state = {}
_DEBUG = False


def _emit(tc, nc, aps):
    import concourse.bass as bass
    from concourse import masks, mybir
    from concourse.tile import add_dep_helper

    f32 = mybir.dt.float32
    bf16 = mybir.dt.bfloat16
    i32 = mybir.dt.int32
    i16 = mybir.dt.int16
    AF = mybir.ActivationFunctionType
    ALU = mybir.AluOpType
    AX = mybir.AxisListType

    Xd, Yd, Td, Sd, Ed, Od = aps
    Xf = Xd.rearrange("b t j -> b (t j)")  # [64, 24576]

    from contextlib import ExitStack

    es = _state["es"] = ExitStack()
    persist = es.enter_context(tc.tile_pool(name="persist", bufs=1))
    xin = es.enter_context(tc.tile_pool(name="xin", bufs=3))
    xinb = es.enter_context(tc.tile_pool(name="xinb", bufs=3))
    sc_ps = es.enter_context(tc.tile_pool(name="sc_ps", bufs=3, space="PSUM"))
    tp_ps = es.enter_context(tc.tile_pool(name="tp_ps", bufs=3, space="PSUM"))
    sh_ps = es.enter_context(tc.tile_pool(name="sh_ps", bufs=1, space="PSUM"))
    rscr = es.enter_context(tc.tile_pool(name="rscr", bufs=2))

    # ---------------- constants ----------------
    ident = persist.tile([128, 64], bf16)   # identity at partitions 64-127
    masks.make_identity(nc, ident[64:128, :])
    identF = persist.tile([128, 64], f32)   # f32 identity for fp32 transposes
    masks.make_identity(nc, identF[64:128, :])
    id32 = persist.tile([32, 32], f32)
    masks.make_identity(nc, id32[:])

    ttab = persist.tile([32, 32], f32)
    nc.sync.dma_start(ttab[:], Td)
    # W64 = [[exp(T), 0], [0, exp(T)^T]]  (bf16, partitions 0-63)
    W64 = persist.tile([64, 64], bf16)
    nc.vector.memset(W64[:], 0.0)
    nc.scalar.activation(W64[0:32, 0:32], ttab[:], AF.Exp)
    # exp(T)^T: transpose at partition 0 (transpose outputs must land at
    # PSUM partition 0), exp there, then DMA to partitions 32-63
    tps2 = sh_ps.tile([32, 32], f32, tag="sp")
    nc.tensor.transpose(tps2[:], ttab[:], id32[:])
    exTT0 = persist.tile([32, 32], bf16)
    nc.scalar.activation(exTT0[:], tps2[:], AF.Exp)
    nc.sync.dma_start(W64[32:64, 32:64], exTT0[:])
    exTT32 = persist.tile([64, 32], bf16)
    nc.sync.dma_start(exTT32[32:64, :], exTT0[:])

    sraw = persist.tile([64, 1], f32)
    nc.sync.dma_start(sraw[0:32, :], Sd)
    nc.sync.dma_start(sraw[32:64, :], Ed)
    expSE = persist.tile([64, 1], f32)  # exp(start) rows 0-31, exp(end) 32-63
    nc.scalar.activation(expSE[:], sraw[:], AF.Exp)

    # sum-selector for renorm: col0 sums rows 0-31 (u), col1 rows 32-63 (y)
    ones2 = persist.tile([64, 2], bf16)
    nc.vector.memset(ones2[:], 0.0)
    nc.vector.memset(ones2[0:32, 0:1], 1.0)
    nc.vector.memset(ones2[32:64, 1:2], 1.0)
    # broadcast selector: row0 -> partitions 0-31, row1 -> partitions 32-63
    # (built via iota+compare; sub-32-aligned partition writes are illegal)
    itc = persist.tile([2, 64], i16)
    nc.gpsimd.iota(itc[:], pattern=[[1, 64]], base=0, channel_multiplier=0)
    itcs = persist.tile([2, 64], i16)
    nc.vector.tensor_scalar(itcs[:], itc[:], 5, None, op0=ALU.logical_shift_right)
    itcsf = persist.tile([2, 64], f32)
    nc.vector.tensor_copy(itcsf[:], itcs[:])
    itp = persist.tile([2, 1], i16)
    nc.gpsimd.iota(itp[:], pattern=[[0, 1]], base=0, channel_multiplier=1)
    itpf = persist.tile([2, 1], f32)
    nc.vector.tensor_copy(itpf[:], itp[:])
    SEL2T = persist.tile([2, 64], bf16)
    nc.vector.tensor_scalar(SEL2T[:], itcsf[:], itpf[:], None, op0=ALU.is_equal)
    ones2c = persist.tile([2, 1], f32)
    nc.vector.memset(ones2c[:], 1.0)
    ones32 = persist.tile([32, 1], f32)
    nc.vector.memset(ones32[:], 1.0)

    # ---------------- numerator setup -------
    # Emissions are one-hot selected on the DVE straight from the scan's
    # chunk tiles (partitions 64-127), so Y lives there too.
    NQ = T // 16  # 48
    Yi = persist.tile([128, T], i32)
    nc.sync.dma_start(Yi[64:128, :], Yd)
    Yfb = persist.tile([128, T], bf16)
    nc.vector.tensor_copy(Yfb[64:128, :], Yi[64:128, :])
    iotaJ = persist.tile([128, 32 * 32], i16)
    nc.gpsimd.iota(iotaJ[64:128, :], pattern=[[0, 32], [1, 32]], base=0,
                   channel_multiplier=0)
    iotaJb = persist.tile([128, 32 * 32], bf16)
    nc.vector.tensor_copy(iotaJb[64:128, :], iotaJ[64:128, :])
    emacc = persist.tile([128, 24], f32)

    # flat Y (partitions 0-63) for the transition pair indices
    Yf_ = persist.tile([64, T], f32)
    nc.sync.dma_start(Yi[0:64, :], Yd)
    nc.vector.tensor_copy(Yf_[:], Yi[0:64, :])
    NP = 800
    pidx = persist.tile([64, NP], f32)
    nc.vector.scalar_tensor_tensor(pidx[:, 0:767], Yf_[:, 0:767], 32.0,
                                   Yf_[:, 1:768], op0=ALU.mult, op1=ALU.add)
    nc.vector.tensor_scalar_add(pidx[:, 767:768], Yf_[:, 0:1], 1024.0)
    nc.vector.tensor_scalar_add(pidx[:, 768:769], Yf_[:, 767:768], 1056.0)
    nc.vector.memset(pidx[:, 769:800], 1088.0)
    pidx16 = persist.tile([64, NP], i16)
    nc.vector.tensor_copy(pidx16[:], pidx[:])
    dpool = es.enter_context(tc.tile_pool(name="dram", bufs=1, space="DRAM"))
    pd = dpool.tile([64, NP], i16)
    pdw = nc.sync.dma_start(pd[:], pidx16[:])
    NPC = NP // 16  # 50
    PIDX = persist.tile([128, 8 * NPC], i16)

    # PIDX wrap DMAs (one per batch, ~50 small packets each) drained a
    # few per scan step on the sync queue
    sync_q = []
    for b in range(B):
        g, tau = b % 8, b // 8

        def _pidxw(b=b, g=g, tau=tau):
            wi = nc.sync.dma_start(
                PIDX[16 * g:16 * g + 16, NPC * tau:NPC * tau + NPC],
                pd[b:b + 1, :].rearrange("a (c p) -> a p c", p=16),
            )
            add_dep_helper(wi.ins, pdw.ins, sync=True,
                           reason="wrap read waits for dram roundtrip write")
        sync_q.append(_pidxw)

    # table: [T flat 1024 | start 32 | end 32 | zeros 4] on 128 partitions
    TTAB = persist.tile([128, 1092], f32)
    nc.gpsimd.memset(TTAB[0:1, :], 0.0)
    nc.sync.dma_start(TTAB[0:1, 0:1024], Td.rearrange("i j -> (i j)"))
    nc.sync.dma_start(TTAB[0:1, 1024:1056], Sd)
    nc.sync.dma_start(TTAB[0:1, 1056:1088], Ed)
    nc.gpsimd.partition_broadcast(TTAB[:], TTAB[0:1, :])
    # dummy gather: loads the gpsimd gather library NOW, so its global
    # engine-drain barrier fires during setup instead of mid-scan
    didx = persist.tile([128, 4], i16)
    nc.vector.memset(didx[:], 0)
    dout_g = persist.tile([128, 64], f32)
    nc.gpsimd.ap_gather(dout_g[:], TTAB[:], didx[:],
                        channels=128, num_elems=1092, d=1, num_idxs=64)

    # selection matrix for the per-group transition combine matmuls
    iop = persist.tile([128, 1], i16)
    nc.gpsimd.iota(iop[:], pattern=[[0, 1]], base=0, channel_multiplier=1)
    iog = persist.tile([128, 8], i16)
    nc.gpsimd.iota(iog[:], pattern=[[1, 8]], base=0, channel_multiplier=0)
    iogf = persist.tile([128, 8], f32)
    nc.vector.tensor_copy(iogf[:], iog[:])
    pdiv = persist.tile([128, 1], i16)
    nc.vector.tensor_scalar(pdiv[:], iop[:], 4, None, op0=ALU.logical_shift_right)
    pdivf = persist.tile([128, 1], f32)
    nc.vector.tensor_copy(pdivf[:], pdiv[:])
    SELe = persist.tile([128, 8], f32)
    nc.vector.tensor_scalar(SELe[:], iogf[:], pdivf[:], None, op0=ALU.is_equal)
    SELt = persist.tile([128, 8], f32)
    nc.vector.tensor_scalar_mul(SELt[:], SELe[:], 1.0 / 16.0)

    tpart = persist.tile([128, 8], f32)

    # ---------------- emission pipeline ----------------
    # paired bf16 exp(X) chunk staging: pair r cols [64r',64r'+32) =
    # exp(x_r), [64r'+32,64r'+64) = exp(x_{767-r}) (r' chunk-local);
    # partitions 64-127 (batch)
    CHT = {}  # chunk i -> staged paired bf16 tile

    XCH = {}  # chunk i -> (fwd tile, bwd tile, lo, blo) for the emission pass

    def chunk_prep_head(i):
        # DMA + exp-cast for chunk i (pairs 32i..32i+31)
        lo = CH * i
        xp = xinb.tile([128, CH * 64], bf16)
        xpv = xp[64:128, :].rearrange("p (r h j) -> p r h j", h=2, j=32)
        CHT[i] = xp
        xf = xin.tile([128, CH * 32], f32)
        nc.sync.dma_start(xf[64:128, :], Xf[:, 32 * lo:32 * (lo + CH)])
        nc.scalar.activation(
            xpv[:, :, 0, :],
            xf[64:128, :].rearrange("p (t j) -> p t j", j=32), AF.Exp)
        xb = xin.tile([128, CH * 32], f32)
        blo = T - lo - CH  # covers t' = blo .. blo+CH-1 (= 767-r desc)
        nc.sync.dma_start(xb[64:128, :], Xf[:, 32 * blo:32 * (blo + CH)])
        # reversed read: pair r = 767-t' ascends as t' descends
        nc.scalar.activation(
            xpv[:, :, 1, :],
            xb[64:128, :].rearrange("p (t j) -> p t j", j=32)[:, ::-1, :],
            AF.Exp)
        XCH[i] = (xf, xb, lo, blo)

    # transposes land in 16-pair PSUM block tiles; one ACT copy per full
    # block moves it to the SBUF EX buffer (so scan muls carry at most one
    # extra semaphore wait per 16 steps)
    EX = persist.tile([64, 64 * NPAIR], bf16)
    BLK = 16
    TPB = {}  # block index -> PSUM block tile

    def emit_pair(r):
        b, k = r // BLK, r % BLK
        if k == 0:
            TPB[b] = tp_ps.tile([64, 64 * BLK], bf16, tag="tp", name=f"tpb_{b}")
        xp = CHT[r // CH]
        rl = r % CH
        nc.tensor.transpose(TPB[b][:, 64 * k:64 * k + 64],
                            xp[64:128, 64 * rl:64 * rl + 64],
                            ident[64:128, :])
        if k == BLK - 1:
            nc.scalar.copy(EX[:, 64 * BLK * b:64 * BLK * (b + 1)], TPB[b][:])
            del TPB[b]

    # ---------------- numerator gathers (merged, all on gpsimd) -------
    # one shared output buffer for both merged gathers (sequential on the
    # gpsimd queue, WAR-serialized by tile deps)
    gtile = persist.tile([128, 8 * NP], f32)

    # transition gather on gpsimd; accums on DVE issued much later (data
    # certainly ready, so they fill scan idle instead of blocking it)

    def gather_trans(tau):
        nc.gpsimd.ap_gather(gtile[:, NP * tau:NP * tau + NP], TTAB[:],
                            PIDX[:, NPC * tau:NPC * tau + NPC],
                            channels=128, num_elems=1092, d=1, num_idxs=NP)

    def accum_tau(tau):
        sl = gtile[:, NP * tau:NP * tau + NP]
        nc.vector.tensor_scalar(sl, sl, 1.0, 0.0, op0=ALU.mult,
                                op1=ALU.add,
                                accum_out=tpart[:, tau:tau + 1])

    # one-hot emission select+accumulate straight from a scan chunk tile:
    # OH = (j == Y[b,t]), then accumulate sum_t X[b,t,Y[b,t]] per batch
    ohscr = persist.tile([128, CH * 32], bf16)

    def emis_chunk(xtile, tlo, col):
        ysl = Yfb[64:128, tlo:tlo + CH]
        yrep = ysl.unsqueeze(2).broadcast_to((64, CH, 32))
        ioj = iotaJb[64:128, :].rearrange("p (t j) -> p t j", j=32)
        oh = ohscr[64:128, :].rearrange("p (t j) -> p t j", j=32)
        nc.vector.tensor_tensor(oh, ioj, yrep, op=ALU.is_equal)
        nc.vector.scalar_tensor_tensor(
            ohscr[64:128, :], ohscr[64:128, :], 1.0, xtile[64:128, :],
            op0=ALU.bypass, op1=ALU.mult,
            accum_out=emacc[64:128, col:col + 1])

    # ---------------- the scan ----------------
    # +1 slot: a final renorm of the last state keeps the combine's
    # chain-product inside the scalar engine's Ln range
    NREN = len(range(RENORM, NPAIR - REN_LAG, RENORM)) + 1
    rst = persist.tile([2, 64 * NREN], bf16)
    ren_slot = [0]

    U2 = [persist.tile([64, 64], bf16, name=f"u2_{k}") for k in range(3)]

    def st(r):
        return U2[r % 3]

    def renorm_a(r):
        # measure sums of both chains on state r.  bf16 scales so the
        # applied factor and the Ln-logged factor are bit-identical.
        sp = sh_ps.tile([2, 64], f32, tag="sp")
        nc.tensor.matmul(sp[:], ones2[:], st(r)[:], tile_position=(0, 0))
        srec = rscr.tile([2, 64], f32)
        nc.vector.reciprocal_approx_fast(srec[:], sp[:])
        m = ren_slot[0]
        ren_slot[0] += 1
        rsl = rst[:, 64 * m:64 * m + 64]
        nc.vector.tensor_copy(rsl, srec[:])
        return rsl

    def renorm_b(rsl):
        rb = sh_ps.tile([64, 64], f32, tag="rb")
        nc.tensor.matmul(rb[:], SEL2T[:], rsl, tile_position=(0, 0))
        return rb

    # prologue: chunk 0 prepped with the first transposes, chunk 1 queued
    chunk_prep_head(0)
    for rr in range(TLEAD + 1):
        emit_pair(rr)
    chunk_prep_head(1)

    # u_0 = exp(start) * ex_0 ; y_767 = exp(end) * ex_767
    nc.vector.tensor_scalar_mul(st(0)[:], EX[:, 0:64], expSE[:, 0:1])

    pend_a = None   # rsl awaiting broadcast
    pend_b = {}     # r -> rb PSUM tile to fuse at step r
    for r in range(1, NPAIR):
        i = r // CH
        if r % CH == 0 and i + 1 < NCHUNK:
            chunk_prep_head(i + 1)
        if r % 8 == 5 and sync_q:
            for _ in range(8):
                if sync_q:
                    sync_q.pop(0)()
        if r >= 16 and (r - 16) % 8 == 0 and (r - 16) // 8 < 8:
            gather_trans((r - 16) // 8)
        if r >= 260 and (r - 260) % 4 == 0 and (r - 260) // 4 < 8:
            accum_tau((r - 260) // 4)
        if r % CH == 16 and r // CH in XCH:
            xf_, xb_, lo_, blo_ = XCH[r // CH]
            emis_chunk(xf_, lo_, 2 * (r // CH))
        if r % CH == 18 and r // CH in XCH:
            xf_, xb_, lo_, blo_ = XCH.pop(r // CH)
            emis_chunk(xb_, blo_, 2 * (r // CH) + 1)
        if r + TLEAD < NPAIR:
            emit_pair(r + TLEAD)
        vp = sc_ps.tile([64, 64], f32, tag="sc")
        nc.tensor.matmul(vp[:], W64[:], st(r - 1)[:], tile_position=(0, 0))
        exs = EX[:, 64 * r:64 * r + 64]
        rb = pend_b.pop(r, None)
        if rb is None:
            nc.vector.tensor_mul(st(r)[:], vp[:], exs)
        else:
            u2t = rscr.tile([64, 64], f32)
            nc.vector.tensor_mul(u2t[:], vp[:], exs)
            nc.vector.tensor_mul(st(r)[:], u2t[:], rb[:])
        if pend_a is not None:
            pend_b[r + REN_LAG - 1] = renorm_b(pend_a)
            pend_a = None
        if r % RENORM == 0 and r + REN_LAG < NPAIR and ren_slot[0] < NREN - 1:
            pend_a = renorm_a(r)

    # ---------------- combine: Z = u_383^T exp(T) y_384 ----------------
    # final renorm: both chains scaled to unit sum (and logged) so the
    # product stays well inside the Ln table range
    rslF = renorm_a(NPAIR - 1)
    rbF = renorm_b(rslF)
    last = persist.tile([64, 64], bf16)
    nc.vector.tensor_mul(last[:], st(NPAIR - 1)[:], rbF[:])
    w383 = sc_ps.tile([32, 64], f32, tag="sc")
    nc.tensor.matmul(w383[:], exTT32[32:64, :], last[32:64, :],
                     tile_position=(32, 0))
    q = persist.tile([32, 64], f32)
    nc.vector.tensor_mul(q[:], w383[:], last[0:32, :])
    combo = sh_ps.tile([1, 64], f32, tag="sp")
    nc.tensor.matmul(combo[:], ones32[:], q[:], tile_position=(0, 0))

    # numerator: emission chunk sums reduced per batch then transposed to
    # [1,64] via the PE; transition group sums folded via SELt matmuls
    if 11 in XCH:
        xf11, xb11, lo11, blo11 = XCH.pop(11)
        emis_chunk(xf11, lo11, 22)
        emis_chunk(xb11, blo11, 23)
    emsum = persist.tile([128, 1], f32)
    nc.vector.tensor_reduce(emsum[64:128, :], emacc[64:128, :], AX.X, ALU.add)
    emT = sc_ps.tile([1, 64], f32, tag="sc")
    nc.tensor.transpose(emT[:], emsum[64:128, :], identF[64:128, :])
    nump = sh_ps.tile([1, 64], f32, tag="rb")
    for tau in range(8):
        sl = nump[0:1, 8 * tau:8 * tau + 8]
        nc.tensor.matmul(sl, tpart[:, tau:tau + 1], SELt[:], start=True,
                         stop=True, tile_position=(0, 0))

    # ---------------- final assembly ----------------
    lncombo = persist.tile([1, 64], f32)
    nc.scalar.activation(lncombo[:], combo[:], AF.Ln)
    lnr = persist.tile([2, 64 * NREN], f32)
    nc.scalar.activation(lnr[:], rst[:], AF.Ln)
    lnrsum = persist.tile([2, 64], f32)
    nc.vector.tensor_reduce(lnrsum[:], lnr[:].rearrange("p (m b) -> p b m", b=64),
                            AX.X, ALU.add)
    lnboth = sh_ps.tile([1, 64], f32, tag="sp")
    nc.tensor.matmul(lnboth[:], ones2c[:], lnrsum[:], tile_position=(0, 0))
    f1 = persist.tile([1, 64], f32)
    nc.vector.tensor_sub(f1[:], nump[:], lncombo[:])
    f1b = persist.tile([1, 64], f32)
    nc.vector.tensor_add(f1b[:], f1[:], emT[:])
    f2 = persist.tile([1, 64], f32)
    nc.vector.tensor_add(f2[:], f1b[:], lnboth[:])
    nc.sync.dma_start(Od, f2[:])

    if _DEBUG:
        def dout(name, ap):
            d = nc.dram_tensor(name, list(ap.shape), ap.dtype,
                               kind="ExternalOutput").ap()
            nc.sync.dma_start(d, ap)
        dout("d_u2", last[:]); dout("d_rst", rst[:]); dout("d_q", q[:])
        dout("d_empart", empart[:]); dout("d_tpart", tpart[:])
        dout("d_f1", f1[:]); dout("d_lnrsum", lnrsum[:])

    es.close()


def _build():
    import concourse.tile as tile
    from concourse import bacc, mybir

    f32 = mybir.dt.float32
    i32 = mybir.dt.int32

    nc = bacc.Bacc("TRN2", target_bir_lowering=False, debug=False,
                   enable_asserts=False, num_devices=NCORES)
    Xd = nc.dram_tensor("x", [B, T, NTAG], f32, kind="ExternalInput").ap()
    Yd = nc.dram_tensor("y", [B, T], i32, kind="ExternalInput").ap()
    Td = nc.dram_tensor("t", [NTAG, NTAG], f32, kind="ExternalInput").ap()
    Sd = nc.dram_tensor("s", [NTAG], f32, kind="ExternalInput").ap()
    Ed = nc.dram_tensor("e", [NTAG], f32, kind="ExternalInput").ap()
    Od = nc.dram_tensor("o", [B], f32, kind="ExternalOutput").ap()
    with tile.TileContext(nc) as tc:
        _emit(tc, nc, (Xd, Yd, Td, Sd, Ed, Od))
    nc.compile()
    return nc


def _numpy_fallback(X, Y, mask, transition, start_trans, end_trans):
    X = np.asarray(X, np.float64)
    Y = np.asarray(Y, np.int64)
    m = np.asarray(mask, bool)
    Tm = np.asarray(transition, np.float64)
    st = np.asarray(start_trans, np.float64)
    en = np.asarray(end_trans, np.float64)
    bs, sl, nt = X.shape
    rb = np.arange(bs)
    mf = m.astype(np.float64)
    score = st[Y[:, 0]] + X[rb, 0, Y[:, 0]]
    emit = np.take_along_axis(X[:, 1:], Y[:, 1:, None], axis=2)[..., 0]
    tr = Tm[Y[:, :-1], Y[:, 1:]]
    score = score + np.sum((tr + emit) * mf[:, 1:], axis=1)
    each_len = m.sum(1).astype(np.int64)
    last_tag = Y[rb, each_len - 1]
    score = score + en[last_tag] * mf[rb, each_len - 1]
    alpha = st[None, :] + X[:, 0]
    for t in range(1, sl):
        s = alpha[:, :, None] + Tm[None] + X[:, t][:, None, :]
        mx = s.max(1)
        new = mx + np.log(np.exp(s - mx[:, None, :]).sum(1))
        alpha = np.where(m[:, t][:, None], new, alpha)
    mx = (alpha + en).max(1)
    logZ = mx + np.log(np.exp(alpha + en - mx[:, None]).sum(1))
    return (score - logZ).astype(np.float32)


def kernel(X, Y, mask, transition, start_trans, end_trans):
    X = np.ascontiguousarray(np.asarray(X, dtype=np.float32))
    Yc = np.ascontiguousarray(np.asarray(Y).astype(np.int32))
    Tm = np.ascontiguousarray(np.asarray(transition, dtype=np.float32))
    st = np.ascontiguousarray(np.asarray(start_trans, dtype=np.float32))
    en = np.ascontiguousarray(np.asarray(end_trans, dtype=np.float32))
    mk = np.asarray(mask)

    if X.shape != (BS, T, NTAG) or not bool(mk.all()):
        return _numpy_fallback(X, Y, mask, transition, start_trans, end_trans)

    from concourse import bass_utils

    if "nc" not in _state:
        _state["nc"] = _build()
    nc = _state["nc"]

    in_maps = []
    for c in range(NCORES):
        sl = slice(B * c, B * (c + 1))
        in_maps.append({"x": X[sl], "y": Yc[sl], "t": Tm, "s": st, "e": en})
    res = bass_utils.run_bass_kernel_spmd(nc, in_maps, core_ids=list(range(NCORES)))
    out = np.concatenate([res.results[c]["o"] for c in range(NCORES)])
    return out.astype(np.float32)


if __name__ == "__main__":
    sys.path.insert(0, "/root/problem")
    import reference

    inputs = reference.setup_inputs()
    inputs = {k: np.asarray(v) for k, v in inputs.items()}
    exp = np.asarray(reference.reference(**inputs))
    act = kernel(**inputs)
    err = np.abs(act - exp) / np.maximum(np.abs(exp), 1e-6)
    print("max rel err:", err.max(), "mean:", err.mean())
